# revision 42
# baseline (speedup 1.0000x reference)
"""Distributed multi-head attention (QKV proj + RoPE + softmax attention + out proj)
on 8 TRN2 NeuronCores.

Sharding: tensor-parallel over heads. Core c owns heads (2c, 2c+1):
  - qkv^T = W_c @ x^T for its 384 channels over all 4096 tokens (bf16 matmul)
  - RoPE on q,k (bf16, partition-swap via SBUF-SBUF DMA)
  - scores^T = k @ q^T per (batch, head): both heads' scores go into one
    2-bank PSUM tile (row-packed K=64 matmuls), one exp [128,1024] on ScalarE
  - ctx^T = [v | 1] @ expS^T : M=65 matmul computes context + softmax denominator
    (ones column baked into the transposed-v layout)
  - per-(qt,head) pipelined normalization: approx reciprocal + partition-
    broadcast via a stride-0 DRAM read (the final qt instead broadcasts via a
    ones-matmul on the PE - nothing left for it to block); batch-1 QKV/rope
    work is drip-fed into batch-0's ACT-bound attention; light dummy-matmul
    drip through batch 1 plus a post-collective burst keep the PE HAM-warm
  - exp split across engines: most k-tiles on ACT (exact), a tunable subset
    on DVE via a Schraudolph bitcast exp (int16(round(x*A+B)) viewed as bf16)
  - one AllToAll per batch redistributes ctx head-sharded -> token-sharded
    (256 tok/core/batch); batch-0's collective rides under batch-1 compute
  - out^T = W_out^T.T @ ctx_full^T + b_out in two 256-token halves: the
    batch-0 half runs while the batch-1 AllToAll is in flight

Host side: transposes/shards weights, runs SPMD, gathers [1024, 512] bf16 per
core (256 tokens per batch), converts to fp32, reassembles [2, 2048, 1024].
"""

import numpy as np
import ml_dtypes

import concourse.bass as bass
import concourse.tile as tile
from concourse import bacc, mybir
from concourse.bass_utils import run_bass_kernel_spmd
from concourse.masks import make_identity

BF16 = ml_dtypes.bfloat16
FP8E4 = ml_dtypes.float8_e4m3fn

B, L, D, H, Hd = 2, 2048, 1024, 16, 64
T = B * L              # 4096 tokens
NC = 8                 # cores
HPC = H // NC          # 2 heads per core
TOK = T // NC          # 512 token shard per core
HTOK = TOK // 2        # 256 tokens per (core, batch)
NT = T // 512          # 8 token n-tiles of 512
KT = L // 128          # 16 k-tiles per batch
QT = L // 512          # 4 q-tiles per batch

F32 = mybir.dt.float32
BF = mybir.dt.bfloat16
F8 = mybir.dt.float8e4
I16 = mybir.dt.int16
WQ_SCALE = 16.0        # W_qkv pre-scaled into fp8e4 normal range

# Schraudolph bf16 exp on DVE: bitcast(int16(round(x*EXP_A + EXP_B))) ~ exp(x)
# (centered: multiplicative error within +-3.1%, zero-mean; verified on HW)
EXP_A = 128.0 / float(np.log(2.0))
EXP_B = 16256.0 - 5.513
# kt slots (of 16) whose exp runs on DVE instead of ACT, per batch: batch 0's
# DVE also carries the rope drip, batch 1's is freer
DVE_KTS = {0: frozenset((1, 4, 10, 13)), 1: frozenset((1, 4, 7, 10, 13, 15))}
# fp8 ctx was tried and reverted: es quantization error passes straight
# through softmax to the output (~+5e-3 rel), too close to the 2e-2 gate
DR_KTS = frozenset()
# kt chunks: scores emitted in runs (row-mode PE), ctx in runs (full mode)
CHUNKS = ((0, 1, 2, 3), (4, 5, 6, 7), (8, 9, 10, 11), (12, 13, 14, 15))


def build(debug=False):
    nc = bacc.Bacc(None, target_bir_lowering=False, num_devices=NC)

    xT = nc.dram_tensor("xT", [D, T], F8, kind="ExternalInput")          # x^T, replicated
    wq = nc.dram_tensor("wqkT", [D, 2 * 128], F8, kind="ExternalInput")   # W_{q,k}^T (fp8, scaled)
    wv = nc.dram_tensor("wvT", [D, 128], BF, kind="ExternalInput")        # W_v^T (bf16)
    xTb = nc.dram_tensor("xTb", [D, T], BF, kind="ExternalInput")         # x^T bf16 (v path)
    bq = nc.dram_tensor("bqkv", [128, 3], F32, kind="ExternalInput")      # bias cols q,k,v
    cosT = nc.dram_tensor("cosT", [128, L], BF, kind="ExternalInput")
    sinT = nc.dram_tensor("sinT", [128, L], BF, kind="ExternalInput")     # sign-folded sin
    wo = nc.dram_tensor("woutT", [D, D], BF, kind="ExternalInput")        # W_out^T, replicated
    bo = nc.dram_tensor("bout", [128, NC], F32, kind="ExternalInput")     # bias cols
    out = nc.dram_tensor("out", [D, TOK], BF, kind="ExternalOutput")
    import os
    dbg_on = os.environ.get("KDBG", "0") == "1"
    dbg = (nc.dram_tensor("dbg", [128, 1536], F32, kind="ExternalOutput")
           if dbg_on else None)

    with tile.TileContext(nc) as tc:
        with tc.tile_pool(name="const", bufs=1) as const, \
             tc.tile_pool(name="big", bufs=1) as big, \
             tc.tile_pool(name="rope", bufs=3) as rope, \
             tc.tile_pool(name="es", bufs=10) as esp, \
             tc.tile_pool(name="cu", bufs=12) as cup, \
             tc.tile_pool(name="small", bufs=3) as small, \
             tc.tile_pool(name="psum", bufs=1, space="PSUM") as psum, \
             tc.tile_pool(name="dram", bufs=1, space="DRAM") as dram:

            # ---------------- constants / weights (loaded before x!) ----------
            ident = const.tile([128, 128], BF, tag="ident")
            make_identity(nc, ident[:])
            ones_bc = const.tile([1, 64], BF, tag="ones_bc")
            nc.vector.memset(ones_bc[:], 1.0)
            ones512 = const.tile([1, 512], BF, tag="ones512")
            nc.vector.memset(ones512[:], 1.0)

            bo_sb = const.tile([128, NC], F32, tag="bo")
            # QKV weights in fp8, DoubleRow [Ki, Ko=2, 384] per 256-channel
            # group; first half races the first x chunks in so the first
            # matmul can issue ~6us after kernel start
            w_sb = []

            def wq_src(g):
                # [ki, ko, m] <- wq[256g + ki + 128*ko, m]: the same blocked
                # channel pairing the x-side DMA uses
                wq_ap = wq[:]
                return bass.AP(
                    tensor=wq_ap.tensor, offset=256 * g * 256,
                    ap=[[256, 128], [128 * 256, 2], [1, 256]])

            for g in range(4):
                t = big.tile([128, 2, 2 * 128], F8, tag=f"w{g}", name=f"w{g}")
                if g < 2:
                    nc.sync.dma_start(t[:], wq_src(g))
                w_sb.append(t)
            bq_sb = const.tile([128, 3], F32, tag="bq")
            nc.scalar.dma_start(bq_sb[:], bq[:])
            for g in range(2, 4):
                nc.sync.dma_start(w_sb[g][:], wq_src(g))
            wv_sb = []
            for k in range(8):
                t = big.tile([128, 128], BF, tag=f"wv{k}", name=f"wv{k}")
                nc.sync.dma_start(t[:], wv[128 * k:128 * (k + 1), :])
                wv_sb.append(t)
            cos_sb = const.tile([128, L], BF, tag="cos")
            sin_sb = const.tile([128, L], BF, tag="sin")
            # (trig loads are issued inside stage1_qkv(0), after the first
            # x chunks, split in 4 so no single 512KB transfer blocks)
            wo_sb = [big.tile([128, D], BF, tag=f"wo{k}", name=f"wo_{k}")
                     for k in range(8)]

            qT_sb = big.tile([128, T], BF, tag="qT")
            kT_sb = big.tile([128, T], BF, tag="kT")
            v_sb = big.tile([128, T], BF, tag="v")
            # transposed v with a built-in ones column: [tok%128, blk, head, 65]
            vn_sb = big.tile([128, T // 128, HPC, 65], BF, tag="vn")
            nc.vector.memset(vn_sb[:, :, :, 64:65], 1.0)
            # fp8 DoubleRow variant for paired blocks: [tok%128, blkpair,
            # head, ko, 80] - col 64 is the ones column, 65-79 pad (zeroed)
            vn2_sb = big.tile([128, T // 256, HPC, 2, 80], F8, tag="vn2")
            nc.vector.memset(vn2_sb[:, :, :, :, 64:80], 0.0)
            nc.vector.memset(vn2_sb[:, :, :, :, 64:65], 1.0)

            # one AllToAll per batch: slot j = tokens [256j, 256j+256) of
            # that batch; batch-0's collective rides under batch-1 compute
            a2a_in = [dram.tile([NC, 128, HTOK], BF, tag=f"a2a_in{b}",
                                name=f"a2a_in{b}") for b in range(B)]
            a2a_out = [dram.tile([NC, 128, HTOK], BF, tag=f"a2a_out{b}",
                                 name=f"a2a_out{b}") for b in range(B)]

            # ---------------- per-stage emitters ------------------------------
            _xc_cache = {}

            def stage1_load(n):
                ts = slice(512 * n, 512 * (n + 1))
                xc = []
                for g in range(4):
                    t = rope.tile([128, 2, 512], F8, tag="xc", bufs=12,
                                  name=f"xc_{n}_{g}")
                    if n == 0:  # startup: keep the first x chunks off the
                        q = nc.scalar if g < 2 else nc.gpsimd  # busy sync q
                    else:
                        q = nc.sync
                    xt_ap = xT[:]
                    src8 = bass.AP(
                        tensor=xt_ap.tensor,
                        offset=256 * g * T + 512 * n,
                        ap=[[T, 128], [128 * T, 2], [1, 512]])
                    q.dma_start(t[:], src8)
                    xc.append(t)
                xb = []
                for k in range(8):
                    t = rope.tile([128, 512], BF, tag="xb", bufs=16,
                                  name=f"xb_{n}_{k}")
                    q = (nc.scalar if k < 4 else nc.gpsimd) if n == 0 \
                        else nc.sync
                    q.dma_start(t[:], xTb[128 * k:128 * (k + 1), ts])
                    xb.append(t)
                _xc_cache[n] = (xc, xb)

            _ps_cache = {}

            def stage1_qkv_m_a(n, m):
                """First half of the QKV accumulation for one (n-tile, m)."""
                ps = psum.tile([128, 512], F32, tag="st", bufs=3,
                               name=f"s1_{n}_{m}")
                _ps_cache[(n, m)] = ps
                xc, xb = _xc_cache[n]
                if m < 2:
                    for g in range(2):
                        nc.tensor.matmul(
                            ps[:],
                            w_sb[g][:, :, 128 * m:128 * (m + 1)],
                            xc[g][:],
                            start=(g == 0), stop=False,
                            perf_mode=mybir.MatmulPerfMode.DoubleRow,
                        )
                else:
                    for k in range(4):
                        nc.tensor.matmul(
                            ps[:], wv_sb[k][:], xb[k][:],
                            start=(k == 0), stop=False,
                        )

            def stage1_qkv_m(n, m):
                """Second half of the accumulation; ACT evicts (+bias), rope
                in bf16 split across DVE and GpSimd."""
                ts = slice(512 * n, 512 * (n + 1))
                cs = slice(512 * (n % QT), 512 * (n % QT) + 512)
                ps = _ps_cache.pop((n, m))
                xc, xb = _xc_cache[n]
                if m < 2:
                    for g in range(2, 4):
                        nc.tensor.matmul(
                            ps[:],
                            w_sb[g][:, :, 128 * m:128 * (m + 1)],
                            xc[g][:],
                            start=False, stop=(g == 3),
                            perf_mode=mybir.MatmulPerfMode.DoubleRow,
                        )
                else:
                    for k in range(4, 8):
                        nc.tensor.matmul(
                            ps[:], wv_sb[k][:], xb[k][:],
                            start=False, stop=(k == 7),
                        )
                if m < 2:  # q or k: ACT evicts (+bias) fast to free the
                    # PSUM slot; rope split across DVE and GpSimd
                    dst = qT_sb if m == 0 else kT_sb
                    qb = rope.tile([128, 512], BF, tag="qb", bufs=5,
                                   name=f"qb_{n}_{m}")
                    nc.scalar.activation(
                        qb[:], ps[:],
                        mybir.ActivationFunctionType.Identity,
                        bias=bq_sb[:, m:m + 1],
                        scale=1.0 / (WQ_SCALE * 8.0) if m == 0
                        else 1.0 / WQ_SCALE)
                    qc = rope.tile([128, 512], BF, tag="qc", name=f"qc_{n}_{m}")
                    nc.vector.tensor_tensor(
                        qc[:], qb[:], cos_sb[:, cs], mybir.AluOpType.mult)
                    qs = rope.tile([128, 512], BF, tag="qs", name=f"qs_{n}_{m}")
                    nc.vector.tensor_tensor(
                        qs[:], qb[:], sin_sb[:, cs], mybir.AluOpType.mult)
                    qw = rope.tile([128, 512], BF, tag="qw", name=f"qw_{n}_{m}")
                    for blk in range(4):
                        sb0 = 32 * (blk ^ 1)
                        nc.gpsimd.dma_start(
                            qw[32 * blk:32 * blk + 32, :],
                            qs[sb0:sb0 + 32, :])
                    nc.vector.tensor_tensor(
                        dst[:, ts], qc[:], qw[:], mybir.AluOpType.add)
                else:  # v: bias only, straight to bf16
                    nc.scalar.activation(
                        v_sb[:, ts], ps[:],
                        mybir.ActivationFunctionType.Identity,
                        bias=bq_sb[:, 2:3])

            def stage1_qkv(n):
                stage1_load(n)
                if n == 0:
                    for c in range(4):
                        cs4 = slice(512 * c, 512 * (c + 1))
                        nc.scalar.dma_start(cos_sb[:, cs4], cosT[:, cs4])
                        nc.scalar.dma_start(sin_sb[:, cs4], sinT[:, cs4])
                for m in range(3):
                    stage1_qkv_m_a(n, m)
                    stage1_qkv_m(n, m)

            def stage1_vtr(j):
                """Transpose one 128-token block of v into vn (both heads)."""
                tp = psum.tile([128, 128], BF, tag="st", bufs=3, name=f"tr_{j}")
                nc.tensor.transpose(tp[:], v_sb[:, 128 * j:128 * (j + 1)], ident[:])
                kt = j % KT
                for h in range(HPC):
                    if kt in DR_KTS:
                        nc.vector.tensor_copy(
                            vn2_sb[:, j // 2, h, j % 2, 0:64],
                            tp[:, 64 * h:64 * (h + 1)])
                    else:
                        nc.vector.tensor_copy(
                            vn_sb[:, j, h, 0:64], tp[:, 64 * h:64 * (h + 1)])

            def stage2_open(b, qt):
                return [psum.tile([80, 512], F32, tag=f"ctx{h}", bufs=1,
                                  name=f"ctx_{b}_{qt}_{h}")
                        for h in range(HPC)]

            def stage2_kts(b, qt, ctxs, fill_boundary):
                qsl = slice(2048 * b + 512 * qt, 2048 * b + 512 * qt + 512)

                def emit_ctx(kt, es):
                    blk = 16 * b + kt
                    for h in range(HPC):
                        nc.tensor.matmul(
                            ctxs[h][0:65, :],
                            vn_sb[:, blk, h, :],
                            es[:, 512 * h:512 * (h + 1)],
                            start=(kt == 0), stop=(kt == KT - 1))

                def emit_ctx_pair(kt, es2):
                    bp = (16 * b + kt) // 2
                    for h in range(HPC):
                        nc.tensor.matmul(
                            ctxs[h][:],
                            vn2_sb[:, bp, h, :, :],
                            es2[:, :, 512 * h:512 * (h + 1)],
                            start=(kt == 0), stop=False,
                            perf_mode=mybir.MatmulPerfMode.DoubleRow)

                # chunked emission: runs of score-pairs (64-row PE mode, so
                # next pair's LDWEIGHTS pulls ahead into the idle row group)
                # alternate with runs of ctx matmuls + drip (128-row mode).
                # One chunk of software pipelining: chunk c's ctx is emitted
                # after chunk c+1's scores so exp has a full chunk of slack.
                # Chunks of 3 match the 3 "st" PSUM slots - a scores run
                # never waits on its own chunk's exp evictions.
                prev = []
                es2_cur = [None]
                for ch in CHUNKS:
                    cur = []
                    for kt in ch:
                        ksl = slice(2048 * b + 128 * kt,
                                    2048 * b + 128 * kt + 128)
                        st2 = psum.tile([128, 1024], F32, tag="st", bufs=3,
                                        name=f"st_{b}_{qt}_{kt}")
                        for h in range(HPC):
                            nc.tensor.matmul(
                                st2[:, 512 * h:512 * (h + 1)],
                                kT_sb[64 * h:64 * (h + 1), ksl],
                                qT_sb[64 * h:64 * (h + 1), qsl],
                                start=True, stop=True)
                        if kt in DR_KTS:
                            if kt % 2 == 0:
                                es2_cur[0] = esp.tile(
                                    [128, 2, 1024], F8, tag="es2",
                                    bufs=6, name=f"es2_{b}_{qt}_{kt}")
                            es2 = es2_cur[0]
                            nc.scalar.activation(
                                es2[:, kt % 2, :], st2[:],
                                mybir.ActivationFunctionType.Exp)
                            if kt % 2 == 1:
                                cur.append(("dr", kt - 1, es2))
                        elif kt in DVE_KTS[b]:
                            es = esp.tile([128, 1024], BF, tag="es",
                                          bufs=6, name=f"es_{b}_{qt}_{kt}")
                            nc.vector.tensor_scalar(
                                es[:].bitcast(I16), st2[:], EXP_A, EXP_B,
                                mybir.AluOpType.mult, mybir.AluOpType.add)
                            cur.append(("bf", kt, es))
                        else:
                            es = esp.tile([128, 1024], BF, tag="es",
                                          bufs=6, name=f"es_{b}_{qt}_{kt}")
                            nc.scalar.activation(
                                es[:], st2[:],
                                mybir.ActivationFunctionType.Exp)
                            cur.append(("bf", kt, es))
                    for kind, kt, e in prev:
                        if kind == "dr":
                            emit_ctx_pair(kt, e)
                        else:
                            emit_ctx(kt, e)
                    prev = cur
                    fill_boundary(b, qt, ch[0])
                for kind, kt, e in prev:
                    if kind == "dr":
                        emit_ctx_pair(kt, e)
                    else:
                        emit_ctx(kt, e)

            def a2a_write(b, qt, h, cn):
                # one 3D DMA moves both 256-token halves of cn into their
                # a2a slots (alternating issue queues - each DIRECT2D costs
                # ~600ns on its issuing engine)
                dst_t = a2a_in[b][:]
                slot = 128 * HTOK
                dst = bass.AP(
                    tensor=dst_t.tensor,
                    offset=dst_t.offset + (2 * qt) * slot + (64 * h) * HTOK,
                    ap=[[HTOK, 64], [slot, 2], [1, HTOK]])
                src_t = cn[:]
                src = bass.AP(
                    tensor=src_t.tensor, offset=src_t.offset,
                    ap=[list(src_t.ap[0]), [HTOK, 2], [1, HTOK]])
                (nc.sync if h == 0 else nc.scalar).dma_start(dst, src)

            def ctx_evict(b, qt, ctxs, pe_bcast=False):
                if pe_bcast:
                    # gpsimd-free eviction (batch 1 runs while the batch-0
                    # collective occupies the gpsimd queue): reciprocal
                    # broadcast over 64 partitions via a ones-matmul on the
                    # PE; both heads' chains interleaved
                    pre = {}
                    for h in range(HPC):
                        dn = small.tile([1, 512], F32, tag="dn",
                                        name=f"dnL_{b}_{qt}_{h}", bufs=3)
                        nc.vector.tensor_copy(dn[:], ctxs[h][64:65, :])
                        rc = small.tile([1, 512], F32, tag="rc",
                                        name=f"rcL_{b}_{qt}_{h}", bufs=3)
                        nc.vector.reciprocal_approx_fast(rc[:], dn[:])
                        rcb = small.tile([1, 512], BF, tag="rcb",
                                         name=f"rcbL_{b}_{qt}_{h}", bufs=3)
                        nc.vector.tensor_copy(rcb[:], rc[:])
                        bcp = psum.tile([128, 512], F32, tag=f"ctx{h}",
                                        bufs=1, name=f"bcpL_{b}_{qt}_{h}")
                        nc.tensor.matmul(
                            bcp[0:64, :], ones_bc[:], rcb[:],
                            start=True, stop=True)
                        pre[h] = bcp
                    for h in range(HPC):
                        cu = cup.tile([65, 512], F32, tag="cu",
                                      name=f"cuL_{b}_{qt}_{h}")
                        nc.vector.tensor_copy(cu[:], ctxs[h][0:65, :])
                        cn = small.tile([64, 512], BF, tag="cn", bufs=8,
                                        name=f"cnL_{b}_{qt}_{h}")
                        nc.vector.tensor_tensor(
                            cn[:], cu[0:64, :], pre[h][0:64, :],
                            mybir.AluOpType.mult)
                        a2a_write(b, qt, h, cn)
                    return
                for h in range(HPC):
                    cu = cup.tile([65, 512], F32, tag="cu",
                                  name=f"cu_{b}_{qt}_{h}")
                    nc.vector.tensor_copy(cu[:], ctxs[h][0:65, :])
                    dn = small.tile([1, 512], F32, tag="dn",
                                    name=f"dn_{b}_{qt}_{h}", bufs=3)
                    nc.vector.tensor_copy(dn[:], ctxs[h][64:65, :])
                    rc = small.tile([1, 512], F32, tag="rc",
                                    name=f"rc_{b}_{qt}_{h}", bufs=3)
                    nc.vector.reciprocal_approx_fast(rc[:], dn[:])
                    # broadcast 1/den over 64 partitions via a stride-0
                    # DRAM read; runs entirely off the PE
                    dr = dram.tile([1, 512], F32, tag="dr",
                                   name=f"dr_{b}_{qt}_{h}", bufs=4)
                    nc.gpsimd.dma_start(dr[:], rc[:])
                    bca = small.tile([64, 512], F32, tag="bca",
                                     name=f"bca_{b}_{qt}_{h}", bufs=3)
                    dr_ap = dr[:]
                    bcast_src = bass.AP(
                        tensor=dr_ap.tensor, offset=dr_ap.offset,
                        ap=[[0, 32]] + [list(p) for p in dr_ap.ap])
                    nc.gpsimd.dma_start(bca[0:32, :], bcast_src)
                    nc.sync.dma_start(bca[32:64, :], bcast_src)
                    bca_ap = bca[:]
                    cn = small.tile([64, 512], BF, tag="cn", bufs=8,
                                    name=f"cn_{b}_{qt}_{h}")
                    nc.vector.tensor_tensor(
                        cn[:], cu[0:64, :], bca_ap,
                        mybir.AluOpType.mult)
                    a2a_write(b, qt, h, cn)

            def run_batch(b, fill_boundary):
                for qt in range(QT):
                    ctxs = stage2_open(b, qt)
                    stage2_kts(b, qt, ctxs, fill_boundary)
                    ctx_evict(b, qt, ctxs, pe_bcast=(b == 1))

            # dummy matmuls keep the PE HAM-warm when real work is thin;
            # short 2-matmul groups so a PSUM "st" slot is never held long
            dummy_scr = small.tile([1, 512], F32, tag="dscr", name="dscr",
                                   bufs=1)
            dummy_cnt = [0]

            def emit_dummies(n, w=512):
                for _ in range(n):
                    i = dummy_cnt[0]
                    dummy_cnt[0] += 1
                    if i % 2 == 0:
                        dummy_cnt.append(psum.tile(
                            [128, 512], F32, tag="st", bufs=3,
                            name=f"dmy{i}"))
                    dp = dummy_cnt[-1]
                    nc.tensor.matmul(
                        dp[:, 0:w], ident[:], kT_sb[:, 0:w],
                        start=(i % 2 == 0), stop=(i % 2 == 1))
                    if i % 2 == 1:
                        nc.vector.tensor_copy(
                            dummy_scr[:, 2 * ((i // 2) % 128):
                                      2 * ((i // 2) % 128) + 2],
                            dp[0:1, 0:2])

            # ---------------- emission schedule -------------------------------
            # batch-1 stage-1 work, cut into drip units
            b1_units = []
            for n in range(QT, NT):
                b1_units.append(lambda n=n: stage1_load(n))
                for m in range(3):
                    b1_units.append(lambda n=n, m=m: stage1_qkv_m_a(n, m))
                    b1_units.append(lambda n=n, m=m: stage1_qkv_m(n, m))
                for j in range(4 * n, 4 * n + 4):
                    b1_units.append(lambda j=j: stage1_vtr(j))
            unit_idx = [0]

            # stage 1 for batch 0 (transposes follow each n-tile's v)
            for n in range(QT):
                stage1_qkv(n)
                for j in range(4 * n, 4 * n + 4):
                    stage1_vtr(j)


            # 20 chunk boundaries in batch 0 carry the 44 batch-1 QKV
            # half-units, up to 3 per boundary, inside the full-mode runs
            def fill_b0(b, qt, kt):
                for _ in range(3):
                    if unit_idx[0] < len(b1_units):
                        b1_units[unit_idx[0]]()
                        unit_idx[0] += 1

            if dbg_on:
                dbt = small.tile([128, 1536], F32, tag="dbt", name="dbt")
                nc.vector.tensor_copy(dbt[:, 0:512], qT_sb[:, 0:512])
                nc.vector.tensor_copy(dbt[:, 512:1024], kT_sb[:, 0:512])
                nc.vector.tensor_copy(dbt[:, 1024:1536], v_sb[:, 0:512])
                nc.sync.dma_start(dbg[:], dbt[:])
            run_batch(0, fill_b0)
            # weights for the out projection: NOT on the gpsimd queue (the
            # collectives will occupy it through batch 1)
            for k in range(8):
                (nc.sync if k % 2 else nc.scalar).dma_start(
                    wo_sb[k][:], wo[128 * k:128 * (k + 1), :])
            nc.scalar.dma_start(bo_sb[:], bo[:])
            while unit_idx[0] < len(b1_units):
                b1_units[unit_idx[0]]()
                unit_idx[0] += 1

            # batch-0 AllToAll: trigger now, transfer rides under batch-1.
            # This is the LAST gpsimd work before the second collective -
            # batch 1 is entirely gpsimd-free.
            nc.gpsimd.collective_compute(
                "AllToAll",
                mybir.AluOpType.bypass,
                replica_groups=[list(range(NC))],
                ins=[a2a_in[0].opt()],
                outs=[a2a_out[0].opt()],
            )
            # batch-0 ctx gather: on the gpsimd queue, which the collective
            # just blocked anyway - executes the moment it lands, mid
            # batch-1, without stalling any other queue
            ctxf0_sb = []
            for k in range(8):
                t = big.tile([128, HTOK], BF, tag=f"cf0_{k}",
                             name=f"cf0_{k}")
                nc.gpsimd.dma_start(t[:], a2a_out[0][k, :, :])
                ctxf0_sb.append(t)

            def fill_b1(b, qt, kt):
                # occasional narrow dummy keeps the PE activity monitor at
                # full clock through batch 1
                if kt in (3, 9):
                    emit_dummies(1, w=128)

            run_batch(1, fill_b1)

            nc.gpsimd.collective_compute(
                "AllToAll",
                mybir.AluOpType.bypass,
                replica_groups=[list(range(NC))],
                ins=[a2a_in[1].opt()],
                outs=[a2a_out[1].opt()],
            )

            # ---------------- stage 4: out projection (per batch half) -------
            # batch-0's half runs on the PE while the batch-1 AllToAll is in
            # flight; batch-1's half follows when its data lands. m-outer
            # accumulation into 8 PSUM quarter-bank regions; bias via ACT.
            oslots = [psum.tile([128, 1024], F32, tag="st", bufs=3,
                                name=f"oacc{i}") for i in range(2)]
            accs = [oslots[i // 4][:, 256 * (i % 4):256 * (i % 4) + 256]
                    for i in range(8)]
            for bh in range(2):
                if bh == 0:
                    ctxf_sb = ctxf0_sb
                    dmaq = [nc.sync, nc.scalar]
                else:
                    ctxf_sb = []
                    ldq = [nc.sync, nc.scalar, nc.gpsimd]
                    for k in range(8):
                        t = big.tile([128, HTOK], BF, tag=f"cf1_{k}",
                                     name=f"cf1_{k}")
                        ldq[k % 3].dma_start(t[:], a2a_out[1][k, :, :])
                        ctxf_sb.append(t)
                    dmaq = [nc.sync, nc.gpsimd]
                for m in range(8):
                    for k in range(8):
                        nc.tensor.matmul(
                            accs[m],
                            wo_sb[k][:, 128 * m:128 * (m + 1)],
                            ctxf_sb[k][:],
                            start=(k == 0), stop=(k == 7))
                    os_t = small.tile([128, HTOK], BF, tag="os",
                                      name=f"os_{bh}_{m}", bufs=6)
                    nc.scalar.activation(  # ACT idle here: evict+bias in one
                        os_t[:], accs[m],
                        mybir.ActivationFunctionType.Identity,
                        bias=bo_sb[:, m:m + 1])
                    dmaq[m % 2].dma_start(
                        out[128 * m:128 * (m + 1), HTOK * bh:HTOK * (bh + 1)],
                        os_t[:])
                if bh == 0:
                    # keep the PE activity monitor warm across the remainder
                    # of the batch-1 collective
                    emit_dummies(6, w=128)
            dscr_dr = dram.tile([1, 512], F32, tag="dscr_dr", name="dscr_dr")
            nc.sync.dma_start(dscr_dr[:], dummy_scr[:])

    nc.compile()
    return nc


_NC_CACHE = None


def _get_nc():
    global _NC_CACHE
    if _NC_CACHE is None:
        _NC_CACHE = build()
    return _NC_CACHE


def _host_prep(x, W_qkv, b_qkv, W_out, b_out):
    x = np.asarray(x, dtype=np.float32)
    W_qkv = np.asarray(W_qkv, dtype=np.float32)
    b_qkv = np.asarray(b_qkv, dtype=np.float32)
    W_out = np.asarray(W_out, dtype=np.float32)
    b_out = np.asarray(b_out, dtype=np.float32)

    scale = 1.0 / np.sqrt(Hd)
    xTb = np.ascontiguousarray(x.reshape(T, D).T).astype(BF16)
    xT = xTb.astype(FP8E4)

    # rope tables (token position within batch), channel-transposed + sign-folded
    inv_freq = 1.0 / (10000.0 ** (np.arange(0, Hd, 2, dtype=np.float32) / Hd))  # [32]
    t_pos = np.arange(L, dtype=np.float32)
    freqs = np.outer(t_pos, inv_freq)                       # [L, 32]
    emb = np.concatenate([freqs, freqs], axis=1)            # [L, 64]
    cos_t = np.cos(emb).T.astype(np.float32)                # [64, L]
    sin_t = np.sin(emb).T.astype(np.float32)                # [64, L]
    sin2 = sin_t.copy()
    sin2[32:, :] *= -1.0                                    # s''[d] = +sin d<32, -sin d>=32
    cosT = np.ascontiguousarray(np.tile(cos_t, (2, 1))).astype(BF16)  # [128, L]
    sinT = np.ascontiguousarray(np.tile(sin2, (2, 1))).astype(BF16)

    woutT = np.ascontiguousarray(W_out.T).astype(BF16)      # [D, D]
    bo_sb = np.ascontiguousarray(b_out.reshape(NC, 128).T.copy()).astype(np.float32)  # [128, 8]

    in_maps = []
    for c in range(NC):
        r = slice(128 * c, 128 * (c + 1))
        Wq = W_qkv[0 * D:1 * D][r] * (scale * 8.0)  # x8 keeps fp8 normal;
        # the q eviction unscales by WQ_SCALE*8
        Wk = W_qkv[1 * D:2 * D][r]
        Wv = W_qkv[2 * D:3 * D][r]
        Wc = np.concatenate([Wq, Wk], axis=0)               # [256, 1024]
        WcT = np.ascontiguousarray(Wc.T * WQ_SCALE).astype(FP8E4)  # [1024, 256]
        WvT = np.ascontiguousarray(Wv.T).astype(BF16)       # [1024, 128]
        bq_c = np.stack([
            b_qkv[0 * D:1 * D][r] * scale,
            b_qkv[1 * D:2 * D][r],
            b_qkv[2 * D:3 * D][r],
        ], axis=1).astype(np.float32)                       # [128, 3]
        in_maps.append({
            "xT": xT,
            "xTb": xTb,
            "wqkT": WcT,
            "wvT": WvT,
            "bqkv": np.ascontiguousarray(bq_c),
            "cosT": cosT,
            "sinT": sinT,
            "woutT": woutT,
            "bout": bo_sb,
        })
    return in_maps


def kernel_run(inputs, trace=False, tmpdir=None):
    nc = _get_nc()
    in_maps = _host_prep(**inputs)
    res = run_bass_kernel_spmd(
        nc, in_maps, list(range(NC)), trace=trace, tmpdir=tmpdir)
    # core c returns [D, 512]: cols 0-255 = batch-0 tokens [256c, 256c+256),
    # cols 256-511 = the same token range of batch 1
    outT = np.empty((D, T), dtype=np.float32)
    for c in range(NC):
        o = np.asarray(res.results[c]["out"]).astype(np.float32)
        outT[:, HTOK * c:HTOK * (c + 1)] = o[:, :HTOK]
        outT[:, L + HTOK * c:L + HTOK * (c + 1)] = o[:, HTOK:]
    out = np.ascontiguousarray(outT.T).reshape(B, L, D)
    return out, res


def kernel(**inputs):
    out, _ = kernel_run(inputs, trace=False)
    return out



# revision 43
# speedup vs baseline: 1.0296x; 1.0296x over previous
"""Distributed multi-head attention (QKV proj + RoPE + softmax attention + out proj)
on 8 TRN2 NeuronCores.

Sharding: tensor-parallel over heads. Core c owns heads (2c, 2c+1):
  - qkv^T = W_c @ x^T for its 384 channels over all 4096 tokens (bf16 matmul)
  - RoPE on q,k (bf16, partition-swap via SBUF-SBUF DMA)
  - scores^T = k @ q^T per (batch, head): both heads' scores go into one
    2-bank PSUM tile (row-packed K=64 matmuls), one exp [128,1024] on ScalarE
  - ctx^T = [v | 1] @ expS^T : M=65 matmul computes context + softmax denominator
    (ones column baked into the transposed-v layout)
  - per-(qt,head) pipelined normalization: approx reciprocal + partition-
    broadcast via a stride-0 DRAM read (the final qt instead broadcasts via a
    ones-matmul on the PE - nothing left for it to block); batch-1 QKV/rope
    work is drip-fed into batch-0's ACT-bound attention; light dummy-matmul
    drip through batch 1 plus a post-collective burst keep the PE HAM-warm
  - exp split across engines: most k-tiles on ACT (exact), a tunable subset
    on DVE via a Schraudolph bitcast exp (int16(round(x*A+B)) viewed as bf16)
  - one AllToAll per batch redistributes ctx head-sharded -> token-sharded
    (256 tok/core/batch); batch-0's collective rides under batch-1 compute
  - out^T = W_out^T.T @ ctx_full^T + b_out in two 256-token halves: the
    batch-0 half runs while the batch-1 AllToAll is in flight

Host side: transposes/shards weights, runs SPMD, gathers [1024, 512] bf16 per
core (256 tokens per batch), converts to fp32, reassembles [2, 2048, 1024].
"""

import numpy as np
import ml_dtypes

import concourse.bass as bass
import concourse.tile as tile
from concourse import bacc, mybir
from concourse.bass_utils import run_bass_kernel_spmd
from concourse.masks import make_identity

BF16 = ml_dtypes.bfloat16
FP8E4 = ml_dtypes.float8_e4m3fn

B, L, D, H, Hd = 2, 2048, 1024, 16, 64
T = B * L              # 4096 tokens
NC = 8                 # cores
HPC = H // NC          # 2 heads per core
TOK = T // NC          # 512 token shard per core
HTOK = TOK // 2        # 256 tokens per (core, batch)
NT = T // 512          # 8 token n-tiles of 512
KT = L // 128          # 16 k-tiles per batch
QT = L // 512          # 4 q-tiles per batch

F32 = mybir.dt.float32
BF = mybir.dt.bfloat16
F8 = mybir.dt.float8e4
I16 = mybir.dt.int16
WQ_SCALE = 16.0        # W_qkv pre-scaled into fp8e4 normal range

# Schraudolph bf16 exp on DVE: bitcast(int16(round(x*EXP_A + EXP_B))) ~ exp(x)
# (centered: multiplicative error within +-3.1%, zero-mean; verified on HW)
EXP_A = 128.0 / float(np.log(2.0))
EXP_B = 16256.0 - 5.513
# kt slots (of 16) whose exp runs on DVE instead of ACT, per batch: batch 0's
# DVE also carries the rope drip, batch 1's is freer
DVE_KTS = {0: frozenset((1, 4, 10, 13)), 1: frozenset((1, 4, 7, 10, 13, 15))}
# fp8 ctx was tried and reverted: es quantization error passes straight
# through softmax to the output (~+5e-3 rel), too close to the 2e-2 gate
DR_KTS = frozenset()
# kt chunks: scores emitted in runs (row-mode PE), ctx in runs (full mode)
CHUNKS = ((0, 1, 2, 3), (4, 5, 6, 7), (8, 9, 10, 11), (12, 13, 14, 15))


def build(debug=False):
    nc = bacc.Bacc(None, target_bir_lowering=False, num_devices=NC)

    xT = nc.dram_tensor("xT", [D, T], F8, kind="ExternalInput")          # x^T, replicated
    wq = nc.dram_tensor("wqkT", [D, 2 * 128], F8, kind="ExternalInput")   # W_{q,k}^T (fp8, scaled)
    wv = nc.dram_tensor("wvT", [D, 128], BF, kind="ExternalInput")        # W_v^T (bf16)
    xTb = nc.dram_tensor("xTb", [D, T], BF, kind="ExternalInput")         # x^T bf16 (v path)
    bq = nc.dram_tensor("bqkv", [128, 3], F32, kind="ExternalInput")      # bias cols q,k,v
    cosT = nc.dram_tensor("cosT", [128, L], BF, kind="ExternalInput")
    sinT = nc.dram_tensor("sinT", [128, L], BF, kind="ExternalInput")     # sign-folded sin
    wo = nc.dram_tensor("woutT", [D, D], BF, kind="ExternalInput")        # W_out^T, replicated
    bo = nc.dram_tensor("bout", [128, NC], F32, kind="ExternalInput")     # bias cols
    out = nc.dram_tensor("out", [D, TOK], BF, kind="ExternalOutput")
    import os
    dbg_on = os.environ.get("KDBG", "0") == "1"
    dbg = (nc.dram_tensor("dbg", [128, 1536], F32, kind="ExternalOutput")
           if dbg_on else None)

    with tile.TileContext(nc) as tc:
        with tc.tile_pool(name="const", bufs=1) as const, \
             tc.tile_pool(name="big", bufs=1) as big, \
             tc.tile_pool(name="rope", bufs=3) as rope, \
             tc.tile_pool(name="es", bufs=10) as esp, \
             tc.tile_pool(name="cu", bufs=12) as cup, \
             tc.tile_pool(name="small", bufs=3) as small, \
             tc.tile_pool(name="psum", bufs=1, space="PSUM") as psum, \
             tc.tile_pool(name="dram", bufs=1, space="DRAM") as dram:

            # ---------------- constants / weights (loaded before x!) ----------
            ident = const.tile([128, 128], BF, tag="ident")
            make_identity(nc, ident[:])
            ones_bc = const.tile([1, 64], BF, tag="ones_bc")
            nc.vector.memset(ones_bc[:], 1.0)
            ones512 = const.tile([1, 512], BF, tag="ones512")
            nc.vector.memset(ones512[:], 1.0)

            bo_sb = const.tile([128, NC], F32, tag="bo")
            # QKV weights in fp8, DoubleRow [Ki, Ko=2, 384] per 256-channel
            # group; first half races the first x chunks in so the first
            # matmul can issue ~6us after kernel start
            w_sb = []

            def wq_src(g):
                # [ki, ko, m] <- wq[256g + ki + 128*ko, m]: the same blocked
                # channel pairing the x-side DMA uses
                wq_ap = wq[:]
                return bass.AP(
                    tensor=wq_ap.tensor, offset=256 * g * 256,
                    ap=[[256, 128], [128 * 256, 2], [1, 256]])

            for g in range(4):
                t = big.tile([128, 2, 2 * 128], F8, tag=f"w{g}", name=f"w{g}")
                if g < 2:
                    nc.sync.dma_start(t[:], wq_src(g))
                w_sb.append(t)
            bq_sb = const.tile([128, 3], F32, tag="bq")
            nc.scalar.dma_start(bq_sb[:], bq[:])
            for g in range(2, 4):
                nc.sync.dma_start(w_sb[g][:], wq_src(g))
            wv_sb = []
            for k in range(8):
                t = big.tile([128, 128], BF, tag=f"wv{k}", name=f"wv{k}")
                nc.sync.dma_start(t[:], wv[128 * k:128 * (k + 1), :])
                wv_sb.append(t)
            cos_sb = const.tile([128, L], BF, tag="cos")
            sin_sb = const.tile([128, L], BF, tag="sin")
            # (trig loads are issued inside stage1_qkv(0), after the first
            # x chunks, split in 4 so no single 512KB transfer blocks)
            wo_sb = [big.tile([128, D], BF, tag=f"wo{k}", name=f"wo_{k}")
                     for k in range(8)]

            qT_sb = big.tile([128, T], BF, tag="qT")
            kT_sb = big.tile([128, T], BF, tag="kT")
            v_sb = big.tile([128, T], BF, tag="v")
            # transposed v with a built-in ones column: [tok%128, blk, head, 65]
            vn_sb = big.tile([128, T // 128, HPC, 65], BF, tag="vn")
            nc.vector.memset(vn_sb[:, :, :, 64:65], 1.0)
            # fp8 DoubleRow variant for paired blocks: [tok%128, blkpair,
            # head, ko, 80] - col 64 is the ones column, 65-79 pad (zeroed)
            vn2_sb = big.tile([128, T // 256, HPC, 2, 80], F8, tag="vn2")
            nc.vector.memset(vn2_sb[:, :, :, :, 64:80], 0.0)
            nc.vector.memset(vn2_sb[:, :, :, :, 64:65], 1.0)

            # one AllToAll per batch: slot j = tokens [256j, 256j+256) of
            # that batch; batch-0's collective rides under batch-1 compute
            a2a_in = [dram.tile([NC, 128, HTOK], BF, tag=f"a2a_in{b}",
                                name=f"a2a_in{b}") for b in range(B)]
            a2a_out = [dram.tile([NC, 128, HTOK], BF, tag=f"a2a_out{b}",
                                 name=f"a2a_out{b}") for b in range(B)]

            # ---------------- per-stage emitters ------------------------------
            _xc_cache = {}

            def stage1_load(n):
                ts = slice(512 * n, 512 * (n + 1))
                xc = []
                for g in range(4):
                    t = rope.tile([128, 2, 512], F8, tag="xc", bufs=12,
                                  name=f"xc_{n}_{g}")
                    if n == 0:  # startup: keep the first x chunks off the
                        q = nc.scalar if g < 2 else nc.gpsimd  # busy sync q
                    else:
                        q = nc.sync
                    xt_ap = xT[:]
                    src8 = bass.AP(
                        tensor=xt_ap.tensor,
                        offset=256 * g * T + 512 * n,
                        ap=[[T, 128], [128 * T, 2], [1, 512]])
                    q.dma_start(t[:], src8)
                    xc.append(t)
                xb = []
                for k in range(8):
                    t = rope.tile([128, 512], BF, tag="xb", bufs=16,
                                  name=f"xb_{n}_{k}")
                    q = (nc.scalar if k < 4 else nc.gpsimd) if n == 0 \
                        else nc.sync
                    q.dma_start(t[:], xTb[128 * k:128 * (k + 1), ts])
                    xb.append(t)
                _xc_cache[n] = (xc, xb)

            _ps_cache = {}

            def stage1_qkv_m_a(n, m):
                """First half of the QKV accumulation for one (n-tile, m)."""
                ps = psum.tile([128, 512], F32, tag="st", bufs=3,
                               name=f"s1_{n}_{m}")
                _ps_cache[(n, m)] = ps
                xc, xb = _xc_cache[n]
                if m < 2:
                    for g in range(2):
                        nc.tensor.matmul(
                            ps[:],
                            w_sb[g][:, :, 128 * m:128 * (m + 1)],
                            xc[g][:],
                            start=(g == 0), stop=False,
                            perf_mode=mybir.MatmulPerfMode.DoubleRow,
                        )
                else:
                    for k in range(4):
                        nc.tensor.matmul(
                            ps[:], wv_sb[k][:], xb[k][:],
                            start=(k == 0), stop=False,
                        )

            def stage1_qkv_m(n, m):
                """Second half of the accumulation; ACT evicts (+bias), rope
                in bf16 split across DVE and GpSimd."""
                ts = slice(512 * n, 512 * (n + 1))
                cs = slice(512 * (n % QT), 512 * (n % QT) + 512)
                ps = _ps_cache.pop((n, m))
                xc, xb = _xc_cache[n]
                if m < 2:
                    for g in range(2, 4):
                        nc.tensor.matmul(
                            ps[:],
                            w_sb[g][:, :, 128 * m:128 * (m + 1)],
                            xc[g][:],
                            start=False, stop=(g == 3),
                            perf_mode=mybir.MatmulPerfMode.DoubleRow,
                        )
                else:
                    for k in range(4, 8):
                        nc.tensor.matmul(
                            ps[:], wv_sb[k][:], xb[k][:],
                            start=False, stop=(k == 7),
                        )
                if m < 2:  # q or k: ACT evicts (+bias) fast to free the
                    # PSUM slot; rope split across DVE and GpSimd
                    dst = qT_sb if m == 0 else kT_sb
                    qb = rope.tile([128, 512], BF, tag="qb", bufs=5,
                                   name=f"qb_{n}_{m}")
                    nc.scalar.activation(
                        qb[:], ps[:],
                        mybir.ActivationFunctionType.Identity,
                        bias=bq_sb[:, m:m + 1],
                        scale=1.0 / (WQ_SCALE * 8.0) if m == 0
                        else 1.0 / WQ_SCALE)
                    qc = rope.tile([128, 512], BF, tag="qc", name=f"qc_{n}_{m}")
                    nc.vector.tensor_tensor(
                        qc[:], qb[:], cos_sb[:, cs], mybir.AluOpType.mult)
                    qs = rope.tile([128, 512], BF, tag="qs", name=f"qs_{n}_{m}")
                    nc.vector.tensor_tensor(
                        qs[:], qb[:], sin_sb[:, cs], mybir.AluOpType.mult)
                    qw = rope.tile([128, 512], BF, tag="qw", name=f"qw_{n}_{m}")
                    for blk in range(4):
                        sb0 = 32 * (blk ^ 1)
                        nc.gpsimd.dma_start(
                            qw[32 * blk:32 * blk + 32, :],
                            qs[sb0:sb0 + 32, :])
                    nc.vector.tensor_tensor(
                        dst[:, ts], qc[:], qw[:], mybir.AluOpType.add)
                else:  # v: bias only, straight to bf16
                    nc.scalar.activation(
                        v_sb[:, ts], ps[:],
                        mybir.ActivationFunctionType.Identity,
                        bias=bq_sb[:, 2:3])

            def stage1_qkv(n):
                stage1_load(n)
                if n == 0:
                    # trig tables on the gpsimd queue: idle from ~12us
                    # (after its n0 x share) until the first evict ~42us,
                    # and its descriptor issues block no compute stream
                    for c in range(4):
                        cs4 = slice(512 * c, 512 * (c + 1))
                        nc.gpsimd.dma_start(cos_sb[:, cs4], cosT[:, cs4])
                        nc.gpsimd.dma_start(sin_sb[:, cs4], sinT[:, cs4])
                for m in range(3):
                    stage1_qkv_m_a(n, m)
                    stage1_qkv_m(n, m)

            def stage1_vtr(j):
                """Transpose one 128-token block of v into vn (both heads)."""
                tp = psum.tile([128, 128], BF, tag="st", bufs=3, name=f"tr_{j}")
                nc.tensor.transpose(tp[:], v_sb[:, 128 * j:128 * (j + 1)], ident[:])
                kt = j % KT
                for h in range(HPC):
                    if kt in DR_KTS:
                        nc.vector.tensor_copy(
                            vn2_sb[:, j // 2, h, j % 2, 0:64],
                            tp[:, 64 * h:64 * (h + 1)])
                    else:
                        nc.vector.tensor_copy(
                            vn_sb[:, j, h, 0:64], tp[:, 64 * h:64 * (h + 1)])

            def stage2_open(b, qt):
                return [psum.tile([80, 512], F32, tag=f"ctx{h}", bufs=1,
                                  name=f"ctx_{b}_{qt}_{h}")
                        for h in range(HPC)]

            def stage2_kts(b, qt, ctxs, fill_boundary):
                qsl = slice(2048 * b + 512 * qt, 2048 * b + 512 * qt + 512)

                def emit_ctx(kt, es):
                    blk = 16 * b + kt
                    for h in range(HPC):
                        nc.tensor.matmul(
                            ctxs[h][0:65, :],
                            vn_sb[:, blk, h, :],
                            es[:, 512 * h:512 * (h + 1)],
                            start=(kt == 0), stop=(kt == KT - 1))

                def emit_ctx_pair(kt, es2):
                    bp = (16 * b + kt) // 2
                    for h in range(HPC):
                        nc.tensor.matmul(
                            ctxs[h][:],
                            vn2_sb[:, bp, h, :, :],
                            es2[:, :, 512 * h:512 * (h + 1)],
                            start=(kt == 0), stop=False,
                            perf_mode=mybir.MatmulPerfMode.DoubleRow)

                # chunked emission: runs of score-pairs (64-row PE mode, so
                # next pair's LDWEIGHTS pulls ahead into the idle row group)
                # alternate with runs of ctx matmuls + drip (128-row mode).
                # One chunk of software pipelining: chunk c's ctx is emitted
                # after chunk c+1's scores so exp has a full chunk of slack.
                # Chunks of 3 match the 3 "st" PSUM slots - a scores run
                # never waits on its own chunk's exp evictions.
                prev = []
                es2_cur = [None]
                for ch in CHUNKS:
                    cur = []
                    for kt in ch:
                        ksl = slice(2048 * b + 128 * kt,
                                    2048 * b + 128 * kt + 128)
                        st2 = psum.tile([128, 1024], F32, tag="st", bufs=3,
                                        name=f"st_{b}_{qt}_{kt}")
                        for h in range(HPC):
                            nc.tensor.matmul(
                                st2[:, 512 * h:512 * (h + 1)],
                                kT_sb[64 * h:64 * (h + 1), ksl],
                                qT_sb[64 * h:64 * (h + 1), qsl],
                                start=True, stop=True)
                        if kt in DR_KTS:
                            if kt % 2 == 0:
                                es2_cur[0] = esp.tile(
                                    [128, 2, 1024], F8, tag="es2",
                                    bufs=6, name=f"es2_{b}_{qt}_{kt}")
                            es2 = es2_cur[0]
                            nc.scalar.activation(
                                es2[:, kt % 2, :], st2[:],
                                mybir.ActivationFunctionType.Exp)
                            if kt % 2 == 1:
                                cur.append(("dr", kt - 1, es2))
                        elif kt in DVE_KTS[b]:
                            es = esp.tile([128, 1024], BF, tag="es",
                                          bufs=6, name=f"es_{b}_{qt}_{kt}")
                            nc.vector.tensor_scalar(
                                es[:].bitcast(I16), st2[:], EXP_A, EXP_B,
                                mybir.AluOpType.mult, mybir.AluOpType.add)
                            cur.append(("bf", kt, es))
                        else:
                            es = esp.tile([128, 1024], BF, tag="es",
                                          bufs=6, name=f"es_{b}_{qt}_{kt}")
                            nc.scalar.activation(
                                es[:], st2[:],
                                mybir.ActivationFunctionType.Exp)
                            cur.append(("bf", kt, es))
                    for kind, kt, e in prev:
                        if kind == "dr":
                            emit_ctx_pair(kt, e)
                        else:
                            emit_ctx(kt, e)
                    prev = cur
                    fill_boundary(b, qt, ch[0])
                for kind, kt, e in prev:
                    if kind == "dr":
                        emit_ctx_pair(kt, e)
                    else:
                        emit_ctx(kt, e)

            def a2a_write(b, qt, h, cn):
                # one 3D DMA moves both 256-token halves of cn into their
                # a2a slots (alternating issue queues - each DIRECT2D costs
                # ~600ns on its issuing engine)
                dst_t = a2a_in[b][:]
                slot = 128 * HTOK
                dst = bass.AP(
                    tensor=dst_t.tensor,
                    offset=dst_t.offset + (2 * qt) * slot + (64 * h) * HTOK,
                    ap=[[HTOK, 64], [slot, 2], [1, HTOK]])
                src_t = cn[:]
                src = bass.AP(
                    tensor=src_t.tensor, offset=src_t.offset,
                    ap=[list(src_t.ap[0]), [HTOK, 2], [1, HTOK]])
                (nc.sync if h == 0 else nc.scalar).dma_start(dst, src)

            def ctx_evict(b, qt, ctxs, pe_bcast=False):
                if pe_bcast:
                    # gpsimd-free eviction (batch 1 runs while the batch-0
                    # collective occupies the gpsimd queue): reciprocal
                    # broadcast over 64 partitions via a ones-matmul on the
                    # PE; both heads' chains interleaved
                    pre = {}
                    for h in range(HPC):
                        dn = small.tile([1, 512], F32, tag="dn",
                                        name=f"dnL_{b}_{qt}_{h}", bufs=3)
                        nc.vector.tensor_copy(dn[:], ctxs[h][64:65, :])
                        rc = small.tile([1, 512], F32, tag="rc",
                                        name=f"rcL_{b}_{qt}_{h}", bufs=3)
                        nc.vector.reciprocal_approx_fast(rc[:], dn[:])
                        rcb = small.tile([1, 512], BF, tag="rcb",
                                         name=f"rcbL_{b}_{qt}_{h}", bufs=3)
                        nc.vector.tensor_copy(rcb[:], rc[:])
                        bcp = psum.tile([128, 512], F32, tag=f"ctx{h}",
                                        bufs=1, name=f"bcpL_{b}_{qt}_{h}")
                        nc.tensor.matmul(
                            bcp[0:64, :], ones_bc[:], rcb[:],
                            start=True, stop=True)
                        pre[h] = bcp
                    for h in range(HPC):
                        cu = cup.tile([65, 512], F32, tag="cu",
                                      name=f"cuL_{b}_{qt}_{h}")
                        nc.vector.tensor_copy(cu[:], ctxs[h][0:65, :])
                        cn = small.tile([64, 512], BF, tag="cn", bufs=8,
                                        name=f"cnL_{b}_{qt}_{h}")
                        nc.vector.tensor_tensor(
                            cn[:], cu[0:64, :], pre[h][0:64, :],
                            mybir.AluOpType.mult)
                        a2a_write(b, qt, h, cn)
                    return
                for h in range(HPC):
                    cu = cup.tile([65, 512], F32, tag="cu",
                                  name=f"cu_{b}_{qt}_{h}")
                    nc.vector.tensor_copy(cu[:], ctxs[h][0:65, :])
                    dn = small.tile([1, 512], F32, tag="dn",
                                    name=f"dn_{b}_{qt}_{h}", bufs=3)
                    nc.vector.tensor_copy(dn[:], ctxs[h][64:65, :])
                    rc = small.tile([1, 512], F32, tag="rc",
                                    name=f"rc_{b}_{qt}_{h}", bufs=3)
                    nc.vector.reciprocal_approx_fast(rc[:], dn[:])
                    # broadcast 1/den over 64 partitions via a stride-0
                    # DRAM read; runs entirely off the PE
                    dr = dram.tile([1, 512], F32, tag="dr",
                                   name=f"dr_{b}_{qt}_{h}", bufs=4)
                    nc.gpsimd.dma_start(dr[:], rc[:])
                    bca = small.tile([64, 512], F32, tag="bca",
                                     name=f"bca_{b}_{qt}_{h}", bufs=3)
                    dr_ap = dr[:]
                    bcast_src = bass.AP(
                        tensor=dr_ap.tensor, offset=dr_ap.offset,
                        ap=[[0, 32]] + [list(p) for p in dr_ap.ap])
                    nc.gpsimd.dma_start(bca[0:32, :], bcast_src)
                    nc.sync.dma_start(bca[32:64, :], bcast_src)
                    bca_ap = bca[:]
                    cn = small.tile([64, 512], BF, tag="cn", bufs=8,
                                    name=f"cn_{b}_{qt}_{h}")
                    nc.vector.tensor_tensor(
                        cn[:], cu[0:64, :], bca_ap,
                        mybir.AluOpType.mult)
                    a2a_write(b, qt, h, cn)

            def run_batch(b, fill_boundary):
                for qt in range(QT):
                    ctxs = stage2_open(b, qt)
                    stage2_kts(b, qt, ctxs, fill_boundary)
                    ctx_evict(b, qt, ctxs, pe_bcast=(b == 1))

            # dummy matmuls keep the PE HAM-warm when real work is thin;
            # short 2-matmul groups so a PSUM "st" slot is never held long
            dummy_scr = small.tile([1, 512], F32, tag="dscr", name="dscr",
                                   bufs=1)
            dummy_cnt = [0]

            def emit_dummies(n, w=512):
                for _ in range(n):
                    i = dummy_cnt[0]
                    dummy_cnt[0] += 1
                    if i % 2 == 0:
                        dummy_cnt.append(psum.tile(
                            [128, 512], F32, tag="st", bufs=3,
                            name=f"dmy{i}"))
                    dp = dummy_cnt[-1]
                    nc.tensor.matmul(
                        dp[:, 0:w], ident[:], kT_sb[:, 0:w],
                        start=(i % 2 == 0), stop=(i % 2 == 1))
                    if i % 2 == 1:
                        nc.vector.tensor_copy(
                            dummy_scr[:, 2 * ((i // 2) % 128):
                                      2 * ((i // 2) % 128) + 2],
                            dp[0:1, 0:2])

            # ---------------- emission schedule -------------------------------
            # batch-1 stage-1 work, cut into drip units
            b1_units = []
            for n in range(QT, NT):
                b1_units.append(lambda n=n: stage1_load(n))
                for m in range(3):
                    b1_units.append(lambda n=n, m=m: stage1_qkv_m_a(n, m))
                    b1_units.append(lambda n=n, m=m: stage1_qkv_m(n, m))
                for j in range(4 * n, 4 * n + 4):
                    b1_units.append(lambda j=j: stage1_vtr(j))
            unit_idx = [0]

            # stage 1 for batch 0 (transposes follow each n-tile's v)
            for n in range(QT):
                stage1_qkv(n)
                for j in range(4 * n, 4 * n + 4):
                    stage1_vtr(j)


            # 20 chunk boundaries in batch 0 carry the 44 batch-1 QKV
            # half-units, up to 3 per boundary, inside the full-mode runs
            def fill_b0(b, qt, kt):
                for _ in range(3):
                    if unit_idx[0] < len(b1_units):
                        b1_units[unit_idx[0]]()
                        unit_idx[0] += 1

            if dbg_on:
                dbt = small.tile([128, 1536], F32, tag="dbt", name="dbt")
                nc.vector.tensor_copy(dbt[:, 0:512], qT_sb[:, 0:512])
                nc.vector.tensor_copy(dbt[:, 512:1024], kT_sb[:, 0:512])
                nc.vector.tensor_copy(dbt[:, 1024:1536], v_sb[:, 0:512])
                nc.sync.dma_start(dbg[:], dbt[:])
            run_batch(0, fill_b0)
            # weights for the out projection: NOT on the gpsimd queue (the
            # collectives will occupy it through batch 1)
            for k in range(8):
                (nc.sync if k % 2 else nc.scalar).dma_start(
                    wo_sb[k][:], wo[128 * k:128 * (k + 1), :])
            nc.scalar.dma_start(bo_sb[:], bo[:])
            while unit_idx[0] < len(b1_units):
                b1_units[unit_idx[0]]()
                unit_idx[0] += 1

            # batch-0 AllToAll: trigger now, transfer rides under batch-1.
            # This is the LAST gpsimd work before the second collective -
            # batch 1 is entirely gpsimd-free.
            nc.gpsimd.collective_compute(
                "AllToAll",
                mybir.AluOpType.bypass,
                replica_groups=[list(range(NC))],
                ins=[a2a_in[0].opt()],
                outs=[a2a_out[0].opt()],
            )
            # batch-0 ctx gather: on the gpsimd queue, which the collective
            # just blocked anyway - executes the moment it lands, mid
            # batch-1, without stalling any other queue
            ctxf0_sb = []
            for k in range(8):
                t = big.tile([128, HTOK], BF, tag=f"cf0_{k}",
                             name=f"cf0_{k}")
                nc.gpsimd.dma_start(t[:], a2a_out[0][k, :, :])
                ctxf0_sb.append(t)

            def fill_b1(b, qt, kt):
                # occasional narrow dummy keeps the PE activity monitor at
                # full clock through batch 1
                if kt in (3, 9):
                    emit_dummies(1, w=128)

            run_batch(1, fill_b1)

            nc.gpsimd.collective_compute(
                "AllToAll",
                mybir.AluOpType.bypass,
                replica_groups=[list(range(NC))],
                ins=[a2a_in[1].opt()],
                outs=[a2a_out[1].opt()],
            )

            # ---------------- stage 4: out projection (per batch half) -------
            # batch-0's half runs on the PE while the batch-1 AllToAll is in
            # flight; batch-1's half follows when its data lands. m-outer
            # accumulation into 8 PSUM quarter-bank regions; bias via ACT.
            oslots = [psum.tile([128, 1024], F32, tag="st", bufs=3,
                                name=f"oacc{i}") for i in range(2)]
            accs = [oslots[i // 4][:, 256 * (i % 4):256 * (i % 4) + 256]
                    for i in range(8)]
            for bh in range(2):
                if bh == 0:
                    ctxf_sb = ctxf0_sb
                    dmaq = [nc.sync, nc.scalar]
                else:
                    ctxf_sb = []
                    ldq = [nc.sync, nc.scalar, nc.gpsimd]
                    for k in range(8):
                        t = big.tile([128, HTOK], BF, tag=f"cf1_{k}",
                                     name=f"cf1_{k}")
                        ldq[k % 3].dma_start(t[:], a2a_out[1][k, :, :])
                        ctxf_sb.append(t)
                    dmaq = [nc.sync, nc.gpsimd]
                for m in range(8):
                    for k in range(8):
                        nc.tensor.matmul(
                            accs[m],
                            wo_sb[k][:, 128 * m:128 * (m + 1)],
                            ctxf_sb[k][:],
                            start=(k == 0), stop=(k == 7))
                    os_t = small.tile([128, HTOK], BF, tag="os",
                                      name=f"os_{bh}_{m}", bufs=6)
                    nc.scalar.activation(  # ACT idle here: evict+bias in one
                        os_t[:], accs[m],
                        mybir.ActivationFunctionType.Identity,
                        bias=bo_sb[:, m:m + 1])
                    dmaq[m % 2].dma_start(
                        out[128 * m:128 * (m + 1), HTOK * bh:HTOK * (bh + 1)],
                        os_t[:])
                if bh == 0:
                    # keep the PE activity monitor warm across the remainder
                    # of the batch-1 collective
                    emit_dummies(6, w=128)
            dscr_dr = dram.tile([1, 512], F32, tag="dscr_dr", name="dscr_dr")
            nc.sync.dma_start(dscr_dr[:], dummy_scr[:])

    nc.compile()
    return nc


_NC_CACHE = None


def _get_nc():
    global _NC_CACHE
    if _NC_CACHE is None:
        _NC_CACHE = build()
    return _NC_CACHE


def _host_prep(x, W_qkv, b_qkv, W_out, b_out):
    x = np.asarray(x, dtype=np.float32)
    W_qkv = np.asarray(W_qkv, dtype=np.float32)
    b_qkv = np.asarray(b_qkv, dtype=np.float32)
    W_out = np.asarray(W_out, dtype=np.float32)
    b_out = np.asarray(b_out, dtype=np.float32)

    scale = 1.0 / np.sqrt(Hd)
    xTb = np.ascontiguousarray(x.reshape(T, D).T).astype(BF16)
    xT = xTb.astype(FP8E4)

    # rope tables (token position within batch), channel-transposed + sign-folded
    inv_freq = 1.0 / (10000.0 ** (np.arange(0, Hd, 2, dtype=np.float32) / Hd))  # [32]
    t_pos = np.arange(L, dtype=np.float32)
    freqs = np.outer(t_pos, inv_freq)                       # [L, 32]
    emb = np.concatenate([freqs, freqs], axis=1)            # [L, 64]
    cos_t = np.cos(emb).T.astype(np.float32)                # [64, L]
    sin_t = np.sin(emb).T.astype(np.float32)                # [64, L]
    sin2 = sin_t.copy()
    sin2[32:, :] *= -1.0                                    # s''[d] = +sin d<32, -sin d>=32
    cosT = np.ascontiguousarray(np.tile(cos_t, (2, 1))).astype(BF16)  # [128, L]
    sinT = np.ascontiguousarray(np.tile(sin2, (2, 1))).astype(BF16)

    woutT = np.ascontiguousarray(W_out.T).astype(BF16)      # [D, D]
    bo_sb = np.ascontiguousarray(b_out.reshape(NC, 128).T.copy()).astype(np.float32)  # [128, 8]

    in_maps = []
    for c in range(NC):
        r = slice(128 * c, 128 * (c + 1))
        Wq = W_qkv[0 * D:1 * D][r] * (scale * 8.0)  # x8 keeps fp8 normal;
        # the q eviction unscales by WQ_SCALE*8
        Wk = W_qkv[1 * D:2 * D][r]
        Wv = W_qkv[2 * D:3 * D][r]
        Wc = np.concatenate([Wq, Wk], axis=0)               # [256, 1024]
        WcT = np.ascontiguousarray(Wc.T * WQ_SCALE).astype(FP8E4)  # [1024, 256]
        WvT = np.ascontiguousarray(Wv.T).astype(BF16)       # [1024, 128]
        bq_c = np.stack([
            b_qkv[0 * D:1 * D][r] * scale,
            b_qkv[1 * D:2 * D][r],
            b_qkv[2 * D:3 * D][r],
        ], axis=1).astype(np.float32)                       # [128, 3]
        in_maps.append({
            "xT": xT,
            "xTb": xTb,
            "wqkT": WcT,
            "wvT": WvT,
            "bqkv": np.ascontiguousarray(bq_c),
            "cosT": cosT,
            "sinT": sinT,
            "woutT": woutT,
            "bout": bo_sb,
        })
    return in_maps


def kernel_run(inputs, trace=False, tmpdir=None):
    nc = _get_nc()
    in_maps = _host_prep(**inputs)
    res = run_bass_kernel_spmd(
        nc, in_maps, list(range(NC)), trace=trace, tmpdir=tmpdir)
    # core c returns [D, 512]: cols 0-255 = batch-0 tokens [256c, 256c+256),
    # cols 256-511 = the same token range of batch 1
    outT = np.empty((D, T), dtype=np.float32)
    for c in range(NC):
        o = np.asarray(res.results[c]["out"]).astype(np.float32)
        outT[:, HTOK * c:HTOK * (c + 1)] = o[:, :HTOK]
        outT[:, L + HTOK * c:L + HTOK * (c + 1)] = o[:, HTOK:]
    out = np.ascontiguousarray(outT.T).reshape(B, L, D)
    return out, res


def kernel(**inputs):
    out, _ = kernel_run(inputs, trace=False)
    return out



# revision 44
# speedup vs baseline: 1.0937x; 1.0623x over previous
"""Distributed multi-head attention (QKV proj + RoPE + softmax attention + out proj)
on 8 TRN2 NeuronCores.

Sharding: tensor-parallel over heads. Core c owns heads (2c, 2c+1):
  - qkv^T = W_c @ x^T for its 384 channels over all 4096 tokens (bf16 matmul)
  - RoPE on q,k (bf16, partition-swap via SBUF-SBUF DMA)
  - scores^T = k @ q^T per (batch, head): both heads' scores go into one
    2-bank PSUM tile (row-packed K=64 matmuls), one exp [128,1024] on ScalarE
  - ctx^T = [v | 1] @ expS^T : M=65 matmul computes context + softmax denominator
    (ones column baked into the transposed-v layout)
  - per-(qt,head) pipelined normalization: approx reciprocal + partition-
    broadcast via a stride-0 DRAM read (the final qt instead broadcasts via a
    ones-matmul on the PE - nothing left for it to block); batch-1 QKV/rope
    work is drip-fed into batch-0's ACT-bound attention; light dummy-matmul
    drip through batch 1 plus a post-collective burst keep the PE HAM-warm
  - exp split across engines: most k-tiles on ACT (exact), a tunable subset
    on DVE via a Schraudolph bitcast exp (int16(round(x*A+B)) viewed as bf16)
  - one AllToAll per batch redistributes ctx head-sharded -> token-sharded
    (256 tok/core/batch); batch-0's collective rides under batch-1 compute
  - out^T = W_out^T.T @ ctx_full^T + b_out in two 256-token halves: the
    batch-0 half runs while the batch-1 AllToAll is in flight

Host side: transposes/shards weights, runs SPMD, gathers [1024, 512] bf16 per
core (256 tokens per batch), converts to fp32, reassembles [2, 2048, 1024].
"""

import numpy as np
import ml_dtypes

import concourse.bass as bass
import concourse.tile as tile
from concourse import bacc, mybir
from concourse.bass_utils import run_bass_kernel_spmd
from concourse.masks import make_identity

BF16 = ml_dtypes.bfloat16
FP8E4 = ml_dtypes.float8_e4m3fn

B, L, D, H, Hd = 2, 2048, 1024, 16, 64
T = B * L              # 4096 tokens
NC = 8                 # cores
HPC = H // NC          # 2 heads per core
TOK = T // NC          # 512 token shard per core
HTOK = TOK // 2        # 256 tokens per (core, batch)
NT = T // 512          # 8 token n-tiles of 512
KT = L // 128          # 16 k-tiles per batch
QT = L // 512          # 4 q-tiles per batch

F32 = mybir.dt.float32
BF = mybir.dt.bfloat16
F8 = mybir.dt.float8e4
I16 = mybir.dt.int16
WQ_SCALE = 16.0        # W_qkv pre-scaled into fp8e4 normal range

# Schraudolph bf16 exp on DVE: bitcast(int16(round(x*EXP_A + EXP_B))) ~ exp(x)
# (centered: multiplicative error within +-3.1%, zero-mean; verified on HW)
EXP_A = 128.0 / float(np.log(2.0))
EXP_B = 16256.0 - 5.513
# kt slots (of 16) whose exp runs on DVE instead of ACT, per batch: batch 0's
# DVE also carries the rope drip, batch 1's is freer
DVE_KTS = {0: frozenset((1, 4, 10, 13)), 1: frozenset((1, 4, 7, 10, 13, 15))}
# fp8 ctx was tried and reverted: es quantization error passes straight
# through softmax to the output (~+5e-3 rel), too close to the 2e-2 gate
DR_KTS = frozenset()
# kt chunks: scores emitted in runs (row-mode PE), ctx in runs (full mode)
CHUNKS = ((0, 1, 2, 3), (4, 5, 6, 7), (8, 9, 10, 11), (12, 13, 14, 15))


def build(debug=False):
    nc = bacc.Bacc(None, target_bir_lowering=False, num_devices=NC)

    xT = nc.dram_tensor("xT", [D, T], F8, kind="ExternalInput")          # x^T, replicated
    wq = nc.dram_tensor("wqkT", [D, 2 * 128], F8, kind="ExternalInput")   # W_{q,k}^T (fp8, scaled)
    wv = nc.dram_tensor("wvT", [D, 128], BF, kind="ExternalInput")        # W_v^T (bf16)
    xTb = nc.dram_tensor("xTb", [D, T], BF, kind="ExternalInput")         # x^T bf16 (v path)
    bq = nc.dram_tensor("bqkv", [128, 3], F32, kind="ExternalInput")      # bias cols q,k,v
    cosT = nc.dram_tensor("cosT", [128, L], BF, kind="ExternalInput")
    sinT = nc.dram_tensor("sinT", [128, L], BF, kind="ExternalInput")     # sign-folded sin
    wo = nc.dram_tensor("woutT", [D, D], BF, kind="ExternalInput")        # W_out^T, replicated
    bo = nc.dram_tensor("bout", [128, NC], F32, kind="ExternalInput")     # bias cols
    out = nc.dram_tensor("out", [D, TOK], BF, kind="ExternalOutput")
    import os
    dbg_on = os.environ.get("KDBG", "0") == "1"
    dbg = (nc.dram_tensor("dbg", [128, 1536], F32, kind="ExternalOutput")
           if dbg_on else None)

    with tile.TileContext(nc) as tc:
        with tc.tile_pool(name="const", bufs=1) as const, \
             tc.tile_pool(name="big", bufs=1) as big, \
             tc.tile_pool(name="rope", bufs=3) as rope, \
             tc.tile_pool(name="es", bufs=10) as esp, \
             tc.tile_pool(name="cu", bufs=12) as cup, \
             tc.tile_pool(name="small", bufs=3) as small, \
             tc.tile_pool(name="psum", bufs=1, space="PSUM") as psum, \
             tc.tile_pool(name="dram", bufs=1, space="DRAM") as dram:

            # ---------------- constants / weights (loaded before x!) ----------
            ident = const.tile([128, 128], BF, tag="ident")
            make_identity(nc, ident[:])
            ones_bc = const.tile([1, 64], BF, tag="ones_bc")
            nc.vector.memset(ones_bc[:], 1.0)
            ones512 = const.tile([1, 512], BF, tag="ones512")
            nc.vector.memset(ones512[:], 1.0)

            bo_sb = const.tile([128, NC], F32, tag="bo")
            # QKV weights in fp8, DoubleRow [Ki, Ko=2, 384] per 256-channel
            # group; first half races the first x chunks in so the first
            # matmul can issue ~6us after kernel start
            w_sb = []

            def wq_src(g):
                # [ki, ko, m] <- wq[256g + ki + 128*ko, m]: the same blocked
                # channel pairing the x-side DMA uses
                wq_ap = wq[:]
                return bass.AP(
                    tensor=wq_ap.tensor, offset=256 * g * 256,
                    ap=[[256, 128], [128 * 256, 2], [1, 256]])

            for g in range(4):
                t = big.tile([128, 2, 2 * 128], F8, tag=f"w{g}", name=f"w{g}")
                if g < 2:
                    nc.sync.dma_start(t[:], wq_src(g))
                w_sb.append(t)
            bq_sb = const.tile([128, 3], F32, tag="bq")
            nc.scalar.dma_start(bq_sb[:], bq[:])
            for g in range(2, 4):
                nc.sync.dma_start(w_sb[g][:], wq_src(g))
            wv_sb = []
            for k in range(8):
                t = big.tile([128, 128], BF, tag=f"wv{k}", name=f"wv{k}")
                nc.sync.dma_start(t[:], wv[128 * k:128 * (k + 1), :])
                wv_sb.append(t)
            cos_sb = const.tile([128, L], BF, tag="cos")
            nc.scalar.dma_start(cos_sb[:], cosT[:])
            sin_sb = const.tile([128, L], BF, tag="sin")
            nc.scalar.dma_start(sin_sb[:], sinT[:])
            wo_sb = [big.tile([128, D], BF, tag=f"wo{k}", name=f"wo_{k}")
                     for k in range(8)]

            qT_sb = big.tile([128, T], BF, tag="qT")
            kT_sb = big.tile([128, T], BF, tag="kT")
            v_sb = big.tile([128, T], BF, tag="v")
            # transposed v with a built-in ones column: [tok%128, blk, head, 65]
            vn_sb = big.tile([128, T // 128, HPC, 65], BF, tag="vn")
            nc.vector.memset(vn_sb[:, :, :, 64:65], 1.0)
            # fp8 DoubleRow variant for paired blocks: [tok%128, blkpair,
            # head, ko, 80] - col 64 is the ones column, 65-79 pad (zeroed)
            vn2_sb = big.tile([128, T // 256, HPC, 2, 80], F8, tag="vn2")
            nc.vector.memset(vn2_sb[:, :, :, :, 64:80], 0.0)
            nc.vector.memset(vn2_sb[:, :, :, :, 64:65], 1.0)

            # one AllToAll per batch: slot j = tokens [256j, 256j+256) of
            # that batch; batch-0's collective rides under batch-1 compute
            a2a_in = [dram.tile([NC, 128, HTOK], BF, tag=f"a2a_in{b}",
                                name=f"a2a_in{b}") for b in range(B)]
            a2a_out = [dram.tile([NC, 128, HTOK], BF, tag=f"a2a_out{b}",
                                 name=f"a2a_out{b}") for b in range(B)]

            # ---------------- per-stage emitters ------------------------------
            _xc_cache = {}

            def stage1_load(n):
                ts = slice(512 * n, 512 * (n + 1))
                xc = []
                for g in range(4):
                    t = rope.tile([128, 2, 512], F8, tag="xc", bufs=12,
                                  name=f"xc_{n}_{g}")
                    if n == 0:  # startup: keep the first x chunks off the
                        q = nc.scalar if g < 2 else nc.gpsimd  # busy sync q
                    else:
                        q = nc.sync
                    xt_ap = xT[:]
                    src8 = bass.AP(
                        tensor=xt_ap.tensor,
                        offset=256 * g * T + 512 * n,
                        ap=[[T, 128], [128 * T, 2], [1, 512]])
                    q.dma_start(t[:], src8)
                    xc.append(t)
                xb = []
                for k in range(8):
                    t = rope.tile([128, 512], BF, tag="xb", bufs=16,
                                  name=f"xb_{n}_{k}")
                    q = (nc.scalar if k < 4 else nc.gpsimd) if n == 0 \
                        else nc.sync
                    q.dma_start(t[:], xTb[128 * k:128 * (k + 1), ts])
                    xb.append(t)
                _xc_cache[n] = (xc, xb)

            _ps_cache = {}

            def stage1_qkv_m_a(n, m):
                """First half of the QKV accumulation for one (n-tile, m)."""
                ps = psum.tile([128, 512], F32, tag="st", bufs=3,
                               name=f"s1_{n}_{m}")
                _ps_cache[(n, m)] = ps
                xc, xb = _xc_cache[n]
                if m < 2:
                    for g in range(2):
                        nc.tensor.matmul(
                            ps[:],
                            w_sb[g][:, :, 128 * m:128 * (m + 1)],
                            xc[g][:],
                            start=(g == 0), stop=False,
                            perf_mode=mybir.MatmulPerfMode.DoubleRow,
                        )
                else:
                    for k in range(4):
                        nc.tensor.matmul(
                            ps[:], wv_sb[k][:], xb[k][:],
                            start=(k == 0), stop=False,
                        )

            def stage1_qkv_m(n, m):
                """Second half of the accumulation; ACT evicts (+bias), rope
                in bf16 split across DVE and GpSimd."""
                ts = slice(512 * n, 512 * (n + 1))
                cs = slice(512 * (n % QT), 512 * (n % QT) + 512)
                ps = _ps_cache.pop((n, m))
                xc, xb = _xc_cache[n]
                if m < 2:
                    for g in range(2, 4):
                        nc.tensor.matmul(
                            ps[:],
                            w_sb[g][:, :, 128 * m:128 * (m + 1)],
                            xc[g][:],
                            start=False, stop=(g == 3),
                            perf_mode=mybir.MatmulPerfMode.DoubleRow,
                        )
                else:
                    for k in range(4, 8):
                        nc.tensor.matmul(
                            ps[:], wv_sb[k][:], xb[k][:],
                            start=False, stop=(k == 7),
                        )
                if m < 2:  # q or k: ACT evicts (+bias) fast to free the
                    # PSUM slot; rope split across DVE and GpSimd
                    dst = qT_sb if m == 0 else kT_sb
                    qb = rope.tile([128, 512], BF, tag="qb", bufs=5,
                                   name=f"qb_{n}_{m}")
                    nc.scalar.activation(
                        qb[:], ps[:],
                        mybir.ActivationFunctionType.Identity,
                        bias=bq_sb[:, m:m + 1],
                        scale=1.0 / (WQ_SCALE * 8.0) if m == 0
                        else 1.0 / WQ_SCALE)
                    qc = rope.tile([128, 512], BF, tag="qc", name=f"qc_{n}_{m}")
                    nc.vector.tensor_tensor(
                        qc[:], qb[:], cos_sb[:, cs], mybir.AluOpType.mult)
                    qs = rope.tile([128, 512], BF, tag="qs", name=f"qs_{n}_{m}")
                    nc.vector.tensor_tensor(
                        qs[:], qb[:], sin_sb[:, cs], mybir.AluOpType.mult)
                    qw = rope.tile([128, 512], BF, tag="qw", name=f"qw_{n}_{m}")
                    for blk in range(4):
                        sb0 = 32 * (blk ^ 1)
                        nc.gpsimd.dma_start(
                            qw[32 * blk:32 * blk + 32, :],
                            qs[sb0:sb0 + 32, :])
                    nc.vector.tensor_tensor(
                        dst[:, ts], qc[:], qw[:], mybir.AluOpType.add)
                else:  # v: bias only, straight to bf16
                    nc.scalar.activation(
                        v_sb[:, ts], ps[:],
                        mybir.ActivationFunctionType.Identity,
                        bias=bq_sb[:, 2:3])

            def stage1_qkv(n):
                stage1_load(n)
                for m in range(3):
                    stage1_qkv_m_a(n, m)
                    stage1_qkv_m(n, m)

            def stage1_vtr(j):
                """Transpose one 128-token block of v into vn (both heads)."""
                tp = psum.tile([128, 128], BF, tag="st", bufs=3, name=f"tr_{j}")
                nc.tensor.transpose(tp[:], v_sb[:, 128 * j:128 * (j + 1)], ident[:])
                kt = j % KT
                for h in range(HPC):
                    if kt in DR_KTS:
                        nc.vector.tensor_copy(
                            vn2_sb[:, j // 2, h, j % 2, 0:64],
                            tp[:, 64 * h:64 * (h + 1)])
                    else:
                        nc.vector.tensor_copy(
                            vn_sb[:, j, h, 0:64], tp[:, 64 * h:64 * (h + 1)])

            def stage2_open(b, qt):
                return [psum.tile([80, 512], F32, tag=f"ctx{h}", bufs=1,
                                  name=f"ctx_{b}_{qt}_{h}")
                        for h in range(HPC)]

            def stage2_kts(b, qt, ctxs, fill_boundary):
                qsl = slice(2048 * b + 512 * qt, 2048 * b + 512 * qt + 512)

                def emit_ctx(kt, es):
                    blk = 16 * b + kt
                    for h in range(HPC):
                        nc.tensor.matmul(
                            ctxs[h][0:65, :],
                            vn_sb[:, blk, h, :],
                            es[:, 512 * h:512 * (h + 1)],
                            start=(kt == 0), stop=(kt == KT - 1))

                def emit_ctx_pair(kt, es2):
                    bp = (16 * b + kt) // 2
                    for h in range(HPC):
                        nc.tensor.matmul(
                            ctxs[h][:],
                            vn2_sb[:, bp, h, :, :],
                            es2[:, :, 512 * h:512 * (h + 1)],
                            start=(kt == 0), stop=False,
                            perf_mode=mybir.MatmulPerfMode.DoubleRow)

                # chunked emission: runs of score-pairs (64-row PE mode, so
                # next pair's LDWEIGHTS pulls ahead into the idle row group)
                # alternate with runs of ctx matmuls + drip (128-row mode).
                # One chunk of software pipelining: chunk c's ctx is emitted
                # after chunk c+1's scores so exp has a full chunk of slack.
                # Chunks of 3 match the 3 "st" PSUM slots - a scores run
                # never waits on its own chunk's exp evictions.
                prev = []
                es2_cur = [None]
                for ch in CHUNKS:
                    cur = []
                    for kt in ch:
                        ksl = slice(2048 * b + 128 * kt,
                                    2048 * b + 128 * kt + 128)
                        st2 = psum.tile([128, 1024], F32, tag="st", bufs=3,
                                        name=f"st_{b}_{qt}_{kt}")
                        for h in range(HPC):
                            nc.tensor.matmul(
                                st2[:, 512 * h:512 * (h + 1)],
                                kT_sb[64 * h:64 * (h + 1), ksl],
                                qT_sb[64 * h:64 * (h + 1), qsl],
                                start=True, stop=True)
                        if kt in DR_KTS:
                            if kt % 2 == 0:
                                es2_cur[0] = esp.tile(
                                    [128, 2, 1024], F8, tag="es2",
                                    bufs=6, name=f"es2_{b}_{qt}_{kt}")
                            es2 = es2_cur[0]
                            nc.scalar.activation(
                                es2[:, kt % 2, :], st2[:],
                                mybir.ActivationFunctionType.Exp)
                            if kt % 2 == 1:
                                cur.append(("dr", kt - 1, es2))
                        elif kt in DVE_KTS[b]:
                            es = esp.tile([128, 1024], BF, tag="es",
                                          bufs=6, name=f"es_{b}_{qt}_{kt}")
                            nc.vector.tensor_scalar(
                                es[:].bitcast(I16), st2[:], EXP_A, EXP_B,
                                mybir.AluOpType.mult, mybir.AluOpType.add)
                            cur.append(("bf", kt, es))
                        else:
                            es = esp.tile([128, 1024], BF, tag="es",
                                          bufs=6, name=f"es_{b}_{qt}_{kt}")
                            nc.scalar.activation(
                                es[:], st2[:],
                                mybir.ActivationFunctionType.Exp)
                            cur.append(("bf", kt, es))
                    for kind, kt, e in prev:
                        if kind == "dr":
                            emit_ctx_pair(kt, e)
                        else:
                            emit_ctx(kt, e)
                    prev = cur
                    fill_boundary(b, qt, ch[0])
                for kind, kt, e in prev:
                    if kind == "dr":
                        emit_ctx_pair(kt, e)
                    else:
                        emit_ctx(kt, e)

            def a2a_write(b, qt, h, cn):
                # one 3D DMA moves both 256-token halves of cn into their
                # a2a slots (alternating issue queues - each DIRECT2D costs
                # ~600ns on its issuing engine)
                dst_t = a2a_in[b][:]
                slot = 128 * HTOK
                dst = bass.AP(
                    tensor=dst_t.tensor,
                    offset=dst_t.offset + (2 * qt) * slot + (64 * h) * HTOK,
                    ap=[[HTOK, 64], [slot, 2], [1, HTOK]])
                src_t = cn[:]
                src = bass.AP(
                    tensor=src_t.tensor, offset=src_t.offset,
                    ap=[list(src_t.ap[0]), [HTOK, 2], [1, HTOK]])
                (nc.sync if h == 0 else nc.scalar).dma_start(dst, src)

            def ctx_evict(b, qt, ctxs, pe_bcast=False):
                if pe_bcast:
                    # gpsimd-free eviction (batch 1 runs while the batch-0
                    # collective occupies the gpsimd queue): reciprocal
                    # broadcast over 64 partitions via a ones-matmul on the
                    # PE; both heads' chains interleaved
                    pre = {}
                    for h in range(HPC):
                        dn = small.tile([1, 512], F32, tag="dn",
                                        name=f"dnL_{b}_{qt}_{h}", bufs=3)
                        nc.vector.tensor_copy(dn[:], ctxs[h][64:65, :])
                        rc = small.tile([1, 512], F32, tag="rc",
                                        name=f"rcL_{b}_{qt}_{h}", bufs=3)
                        nc.vector.reciprocal_approx_fast(rc[:], dn[:])
                        rcb = small.tile([1, 512], BF, tag="rcb",
                                         name=f"rcbL_{b}_{qt}_{h}", bufs=3)
                        nc.vector.tensor_copy(rcb[:], rc[:])
                        bcp = psum.tile([128, 512], F32, tag=f"ctx{h}",
                                        bufs=1, name=f"bcpL_{b}_{qt}_{h}")
                        nc.tensor.matmul(
                            bcp[0:64, :], ones_bc[:], rcb[:],
                            start=True, stop=True)
                        pre[h] = bcp
                    for h in range(HPC):
                        cu = cup.tile([65, 512], F32, tag="cu",
                                      name=f"cuL_{b}_{qt}_{h}")
                        nc.vector.tensor_copy(cu[:], ctxs[h][0:65, :])
                        cn = small.tile([64, 512], BF, tag="cn", bufs=8,
                                        name=f"cnL_{b}_{qt}_{h}")
                        nc.vector.tensor_tensor(
                            cn[:], cu[0:64, :], pre[h][0:64, :],
                            mybir.AluOpType.mult)
                        a2a_write(b, qt, h, cn)
                    return
                for h in range(HPC):
                    cu = cup.tile([65, 512], F32, tag="cu",
                                  name=f"cu_{b}_{qt}_{h}")
                    nc.vector.tensor_copy(cu[:], ctxs[h][0:65, :])
                    dn = small.tile([1, 512], F32, tag="dn",
                                    name=f"dn_{b}_{qt}_{h}", bufs=3)
                    nc.vector.tensor_copy(dn[:], ctxs[h][64:65, :])
                    rc = small.tile([1, 512], F32, tag="rc",
                                    name=f"rc_{b}_{qt}_{h}", bufs=3)
                    nc.vector.reciprocal_approx_fast(rc[:], dn[:])
                    # broadcast 1/den over 64 partitions via a stride-0
                    # DRAM read; runs entirely off the PE
                    dr = dram.tile([1, 512], F32, tag="dr",
                                   name=f"dr_{b}_{qt}_{h}", bufs=4)
                    nc.gpsimd.dma_start(dr[:], rc[:])
                    bca = small.tile([64, 512], F32, tag="bca",
                                     name=f"bca_{b}_{qt}_{h}", bufs=3)
                    dr_ap = dr[:]
                    bcast_src = bass.AP(
                        tensor=dr_ap.tensor, offset=dr_ap.offset,
                        ap=[[0, 32]] + [list(p) for p in dr_ap.ap])
                    nc.gpsimd.dma_start(bca[0:32, :], bcast_src)
                    nc.sync.dma_start(bca[32:64, :], bcast_src)
                    bca_ap = bca[:]
                    cn = small.tile([64, 512], BF, tag="cn", bufs=8,
                                    name=f"cn_{b}_{qt}_{h}")
                    nc.vector.tensor_tensor(
                        cn[:], cu[0:64, :], bca_ap,
                        mybir.AluOpType.mult)
                    a2a_write(b, qt, h, cn)

            def run_batch(b, fill_boundary):
                for qt in range(QT):
                    ctxs = stage2_open(b, qt)
                    stage2_kts(b, qt, ctxs, fill_boundary)
                    ctx_evict(b, qt, ctxs, pe_bcast=(b == 1))

            # dummy matmuls keep the PE HAM-warm when real work is thin;
            # short 2-matmul groups so a PSUM "st" slot is never held long
            dummy_scr = small.tile([1, 512], F32, tag="dscr", name="dscr",
                                   bufs=1)
            dummy_cnt = [0]

            def emit_dummies(n, w=512):
                for _ in range(n):
                    i = dummy_cnt[0]
                    dummy_cnt[0] += 1
                    if i % 2 == 0:
                        dummy_cnt.append(psum.tile(
                            [128, 512], F32, tag="st", bufs=3,
                            name=f"dmy{i}"))
                    dp = dummy_cnt[-1]
                    nc.tensor.matmul(
                        dp[:, 0:w], ident[:], kT_sb[:, 0:w],
                        start=(i % 2 == 0), stop=(i % 2 == 1))
                    if i % 2 == 1:
                        nc.vector.tensor_copy(
                            dummy_scr[:, 2 * ((i // 2) % 128):
                                      2 * ((i // 2) % 128) + 2],
                            dp[0:1, 0:2])

            # ---------------- emission schedule -------------------------------
            # batch-1 stage-1 work, cut into drip units
            b1_units = []
            for n in range(QT, NT):
                b1_units.append(lambda n=n: stage1_load(n))
                for m in range(3):
                    b1_units.append(lambda n=n, m=m: stage1_qkv_m_a(n, m))
                    b1_units.append(lambda n=n, m=m: stage1_qkv_m(n, m))
                for j in range(4 * n, 4 * n + 4):
                    b1_units.append(lambda j=j: stage1_vtr(j))
            unit_idx = [0]

            # stage 1 for batch 0 (transposes follow each n-tile's v)
            for n in range(QT):
                stage1_qkv(n)
                for j in range(4 * n, 4 * n + 4):
                    stage1_vtr(j)


            # 20 chunk boundaries in batch 0 carry the 44 batch-1 QKV
            # half-units, up to 3 per boundary, inside the full-mode runs
            def fill_b0(b, qt, kt):
                for _ in range(3):
                    if unit_idx[0] < len(b1_units):
                        b1_units[unit_idx[0]]()
                        unit_idx[0] += 1

            if dbg_on:
                dbt = small.tile([128, 1536], F32, tag="dbt", name="dbt")
                nc.vector.tensor_copy(dbt[:, 0:512], qT_sb[:, 0:512])
                nc.vector.tensor_copy(dbt[:, 512:1024], kT_sb[:, 0:512])
                nc.vector.tensor_copy(dbt[:, 1024:1536], v_sb[:, 0:512])
                nc.sync.dma_start(dbg[:], dbt[:])
            run_batch(0, fill_b0)
            # weights for the out projection: NOT on the gpsimd queue (the
            # collectives will occupy it through batch 1)
            for k in range(8):
                (nc.sync if k % 2 else nc.scalar).dma_start(
                    wo_sb[k][:], wo[128 * k:128 * (k + 1), :])
            nc.scalar.dma_start(bo_sb[:], bo[:])
            while unit_idx[0] < len(b1_units):
                b1_units[unit_idx[0]]()
                unit_idx[0] += 1

            # batch-0 AllToAll: trigger now, transfer rides under batch-1.
            # This is the LAST gpsimd work before the second collective -
            # batch 1 is entirely gpsimd-free.
            nc.gpsimd.collective_compute(
                "AllToAll",
                mybir.AluOpType.bypass,
                replica_groups=[list(range(NC))],
                ins=[a2a_in[0].opt()],
                outs=[a2a_out[0].opt()],
            )
            # batch-0 ctx gather: on the gpsimd queue, which the collective
            # just blocked anyway - executes the moment it lands, mid
            # batch-1, without stalling any other queue
            ctxf0_sb = []
            for k in range(8):
                t = big.tile([128, HTOK], BF, tag=f"cf0_{k}",
                             name=f"cf0_{k}")
                nc.gpsimd.dma_start(t[:], a2a_out[0][k, :, :])
                ctxf0_sb.append(t)

            def fill_b1(b, qt, kt):
                # occasional narrow dummy keeps the PE activity monitor at
                # full clock through batch 1
                if kt in (3, 9):
                    emit_dummies(1, w=128)

            run_batch(1, fill_b1)

            nc.gpsimd.collective_compute(
                "AllToAll",
                mybir.AluOpType.bypass,
                replica_groups=[list(range(NC))],
                ins=[a2a_in[1].opt()],
                outs=[a2a_out[1].opt()],
            )

            # ---------------- stage 4: out projection (per batch half) -------
            # batch-0's half runs on the PE while the batch-1 AllToAll is in
            # flight; batch-1's half follows when its data lands. m-outer
            # accumulation into 8 PSUM quarter-bank regions; bias via ACT.
            oslots = [psum.tile([128, 1024], F32, tag="st", bufs=3,
                                name=f"oacc{i}") for i in range(2)]
            accs = [oslots[i // 4][:, 256 * (i % 4):256 * (i % 4) + 256]
                    for i in range(8)]
            for bh in range(2):
                if bh == 0:
                    ctxf_sb = ctxf0_sb
                    dmaq = [nc.sync, nc.scalar]
                else:
                    ctxf_sb = []
                    ldq = [nc.sync, nc.scalar, nc.gpsimd]
                    for k in range(8):
                        t = big.tile([128, HTOK], BF, tag=f"cf1_{k}",
                                     name=f"cf1_{k}")
                        ldq[k % 3].dma_start(t[:], a2a_out[1][k, :, :])
                        ctxf_sb.append(t)
                    dmaq = [nc.sync, nc.gpsimd]
                for m in range(8):
                    for k in range(8):
                        nc.tensor.matmul(
                            accs[m],
                            wo_sb[k][:, 128 * m:128 * (m + 1)],
                            ctxf_sb[k][:],
                            start=(k == 0), stop=(k == 7))
                    os_t = small.tile([128, HTOK], BF, tag="os",
                                      name=f"os_{bh}_{m}", bufs=6)
                    nc.scalar.activation(  # ACT idle here: evict+bias in one
                        os_t[:], accs[m],
                        mybir.ActivationFunctionType.Identity,
                        bias=bo_sb[:, m:m + 1])
                    dmaq[m % 2].dma_start(
                        out[128 * m:128 * (m + 1), HTOK * bh:HTOK * (bh + 1)],
                        os_t[:])
                if bh == 0:
                    # keep the PE activity monitor warm across the remainder
                    # of the batch-1 collective
                    emit_dummies(6, w=128)
            dscr_dr = dram.tile([1, 512], F32, tag="dscr_dr", name="dscr_dr")
            nc.sync.dma_start(dscr_dr[:], dummy_scr[:])

    nc.compile()
    return nc


_NC_CACHE = None


def _get_nc():
    global _NC_CACHE
    if _NC_CACHE is None:
        _NC_CACHE = build()
    return _NC_CACHE


def _host_prep(x, W_qkv, b_qkv, W_out, b_out):
    x = np.asarray(x, dtype=np.float32)
    W_qkv = np.asarray(W_qkv, dtype=np.float32)
    b_qkv = np.asarray(b_qkv, dtype=np.float32)
    W_out = np.asarray(W_out, dtype=np.float32)
    b_out = np.asarray(b_out, dtype=np.float32)

    scale = 1.0 / np.sqrt(Hd)
    xTb = np.ascontiguousarray(x.reshape(T, D).T).astype(BF16)
    xT = xTb.astype(FP8E4)

    # rope tables (token position within batch), channel-transposed + sign-folded
    inv_freq = 1.0 / (10000.0 ** (np.arange(0, Hd, 2, dtype=np.float32) / Hd))  # [32]
    t_pos = np.arange(L, dtype=np.float32)
    freqs = np.outer(t_pos, inv_freq)                       # [L, 32]
    emb = np.concatenate([freqs, freqs], axis=1)            # [L, 64]
    cos_t = np.cos(emb).T.astype(np.float32)                # [64, L]
    sin_t = np.sin(emb).T.astype(np.float32)                # [64, L]
    sin2 = sin_t.copy()
    sin2[32:, :] *= -1.0                                    # s''[d] = +sin d<32, -sin d>=32
    cosT = np.ascontiguousarray(np.tile(cos_t, (2, 1))).astype(BF16)  # [128, L]
    sinT = np.ascontiguousarray(np.tile(sin2, (2, 1))).astype(BF16)

    woutT = np.ascontiguousarray(W_out.T).astype(BF16)      # [D, D]
    bo_sb = np.ascontiguousarray(b_out.reshape(NC, 128).T.copy()).astype(np.float32)  # [128, 8]

    in_maps = []
    for c in range(NC):
        r = slice(128 * c, 128 * (c + 1))
        Wq = W_qkv[0 * D:1 * D][r] * (scale * 8.0)  # x8 keeps fp8 normal;
        # the q eviction unscales by WQ_SCALE*8
        Wk = W_qkv[1 * D:2 * D][r]
        Wv = W_qkv[2 * D:3 * D][r]
        Wc = np.concatenate([Wq, Wk], axis=0)               # [256, 1024]
        WcT = np.ascontiguousarray(Wc.T * WQ_SCALE).astype(FP8E4)  # [1024, 256]
        WvT = np.ascontiguousarray(Wv.T).astype(BF16)       # [1024, 128]
        bq_c = np.stack([
            b_qkv[0 * D:1 * D][r] * scale,
            b_qkv[1 * D:2 * D][r],
            b_qkv[2 * D:3 * D][r],
        ], axis=1).astype(np.float32)                       # [128, 3]
        in_maps.append({
            "xT": xT,
            "xTb": xTb,
            "wqkT": WcT,
            "wvT": WvT,
            "bqkv": np.ascontiguousarray(bq_c),
            "cosT": cosT,
            "sinT": sinT,
            "woutT": woutT,
            "bout": bo_sb,
        })
    return in_maps


def kernel_run(inputs, trace=False, tmpdir=None):
    nc = _get_nc()
    in_maps = _host_prep(**inputs)
    res = run_bass_kernel_spmd(
        nc, in_maps, list(range(NC)), trace=trace, tmpdir=tmpdir)
    # core c returns [D, 512]: cols 0-255 = batch-0 tokens [256c, 256c+256),
    # cols 256-511 = the same token range of batch 1
    outT = np.empty((D, T), dtype=np.float32)
    for c in range(NC):
        o = np.asarray(res.results[c]["out"]).astype(np.float32)
        outT[:, HTOK * c:HTOK * (c + 1)] = o[:, :HTOK]
        outT[:, L + HTOK * c:L + HTOK * (c + 1)] = o[:, HTOK:]
    out = np.ascontiguousarray(outT.T).reshape(B, L, D)
    return out, res


def kernel(**inputs):
    out, _ = kernel_run(inputs, trace=False)
    return out



# revision 45
# speedup vs baseline: 1.1476x; 1.0493x over previous
"""Distributed multi-head attention (QKV proj + RoPE + softmax attention + out proj)
on 8 TRN2 NeuronCores.

Sharding: tensor-parallel over heads. Core c owns heads (2c, 2c+1):
  - qkv^T = W_c @ x^T for its 384 channels over all 4096 tokens (bf16 matmul)
  - RoPE on q,k (bf16, partition-swap via SBUF-SBUF DMA)
  - scores^T = k @ q^T per (batch, head): both heads' scores go into one
    2-bank PSUM tile (row-packed K=64 matmuls), one exp [128,1024] on ScalarE
  - ctx^T = [v | 1] @ expS^T : M=65 matmul computes context + softmax denominator
    (ones column baked into the transposed-v layout)
  - per-(qt,head) pipelined normalization: approx reciprocal + partition-
    broadcast via a stride-0 DRAM read (the final qt instead broadcasts via a
    ones-matmul on the PE - nothing left for it to block); batch-1 QKV/rope
    work is drip-fed into batch-0's ACT-bound attention; light dummy-matmul
    drip through batch 1 plus a post-collective burst keep the PE HAM-warm
  - exp split across engines: most k-tiles on ACT (exact), a tunable subset
    on DVE via a Schraudolph bitcast exp (int16(round(x*A+B)) viewed as bf16)
  - one AllToAll per batch redistributes ctx head-sharded -> token-sharded
    (256 tok/core/batch); batch-0's collective rides under batch-1 compute
  - out^T = W_out^T.T @ ctx_full^T + b_out in two 256-token halves: the
    batch-0 half runs while the batch-1 AllToAll is in flight

Host side: transposes/shards weights, runs SPMD, gathers [1024, 512] bf16 per
core (256 tokens per batch), converts to fp32, reassembles [2, 2048, 1024].
"""

import numpy as np
import ml_dtypes

import concourse.bass as bass
import concourse.tile as tile
from concourse import bacc, mybir
from concourse.bass_utils import run_bass_kernel_spmd
from concourse.masks import make_identity

BF16 = ml_dtypes.bfloat16
FP8E4 = ml_dtypes.float8_e4m3fn

B, L, D, H, Hd = 2, 2048, 1024, 16, 64
T = B * L              # 4096 tokens
NC = 8                 # cores
HPC = H // NC          # 2 heads per core
TOK = T // NC          # 512 token shard per core
HTOK = TOK // 2        # 256 tokens per (core, batch)
NT = T // 512          # 8 token n-tiles of 512
KT = L // 128          # 16 k-tiles per batch
QT = L // 512          # 4 q-tiles per batch

F32 = mybir.dt.float32
BF = mybir.dt.bfloat16
F8 = mybir.dt.float8e4
I16 = mybir.dt.int16
WQ_SCALE = 16.0        # W_qkv pre-scaled into fp8e4 normal range

# Schraudolph bf16 exp on DVE: bitcast(int16(round(x*EXP_A + EXP_B))) ~ exp(x)
# (centered: multiplicative error within +-3.1%, zero-mean; verified on HW)
EXP_A = 128.0 / float(np.log(2.0))
EXP_B = 16256.0 - 5.513
# kt slots (of 16) whose exp runs on DVE instead of ACT, per batch: batch 0's
# DVE also carries the rope drip, batch 1's is freer
DVE_KTS = {0: frozenset((1, 4, 10, 13)), 1: frozenset((1, 4, 7, 10, 13, 15))}
# fp8 ctx was tried and reverted: es quantization error passes straight
# through softmax to the output (~+5e-3 rel), too close to the 2e-2 gate
DR_KTS = frozenset()
# kt chunks: scores emitted in runs (row-mode PE), ctx in runs (full mode)
CHUNKS = ((0, 1, 2, 3), (4, 5, 6, 7), (8, 9, 10, 11), (12, 13, 14, 15))


def build(debug=False):
    nc = bacc.Bacc(None, target_bir_lowering=False, num_devices=NC)

    xT = nc.dram_tensor("xT", [D, T], F8, kind="ExternalInput")          # x^T, replicated
    wq = nc.dram_tensor("wqkT", [D, 2 * 128], F8, kind="ExternalInput")   # W_{q,k}^T (fp8, scaled)
    wv = nc.dram_tensor("wvT", [D, 128], BF, kind="ExternalInput")        # W_v^T (bf16)
    xTb = nc.dram_tensor("xTb", [D, T], BF, kind="ExternalInput")         # x^T bf16 (v path)
    bq = nc.dram_tensor("bqkv", [128, 3], F32, kind="ExternalInput")      # bias cols q,k,v
    cosT = nc.dram_tensor("cosT", [128, L], BF, kind="ExternalInput")
    sinT = nc.dram_tensor("sinT", [128, L], BF, kind="ExternalInput")     # sign-folded sin
    wo = nc.dram_tensor("woutT", [D, D], BF, kind="ExternalInput")        # W_out^T, replicated
    bo = nc.dram_tensor("bout", [128, NC], F32, kind="ExternalInput")     # bias cols
    out = nc.dram_tensor("out", [D, TOK], BF, kind="ExternalOutput")
    import os
    dbg_on = os.environ.get("KDBG", "0") == "1"
    dbg = (nc.dram_tensor("dbg", [128, 1536], F32, kind="ExternalOutput")
           if dbg_on else None)

    with tile.TileContext(nc) as tc:
        with tc.tile_pool(name="const", bufs=1) as const, \
             tc.tile_pool(name="big", bufs=1) as big, \
             tc.tile_pool(name="rope", bufs=3) as rope, \
             tc.tile_pool(name="es", bufs=10) as esp, \
             tc.tile_pool(name="cu", bufs=12) as cup, \
             tc.tile_pool(name="small", bufs=3) as small, \
             tc.tile_pool(name="psum", bufs=1, space="PSUM") as psum, \
             tc.tile_pool(name="dram", bufs=1, space="DRAM") as dram:

            # ---------------- constants / weights (loaded before x!) ----------
            ident = const.tile([128, 128], BF, tag="ident")
            make_identity(nc, ident[:])
            ones_bc = const.tile([1, 64], BF, tag="ones_bc")
            nc.vector.memset(ones_bc[:], 1.0)
            ones512 = const.tile([1, 512], BF, tag="ones512")
            nc.vector.memset(ones512[:], 1.0)

            bo_sb = const.tile([128, NC], F32, tag="bo")
            # QKV weights in fp8, DoubleRow [Ki, Ko=2, 384] per 256-channel
            # group; first half races the first x chunks in so the first
            # matmul can issue ~6us after kernel start
            w_sb = []

            def wq_src(g):
                # [ki, ko, m] <- wq[256g + ki + 128*ko, m]: the same blocked
                # channel pairing the x-side DMA uses
                wq_ap = wq[:]
                return bass.AP(
                    tensor=wq_ap.tensor, offset=256 * g * 256,
                    ap=[[256, 128], [128 * 256, 2], [1, 256]])

            for g in range(4):
                t = big.tile([128, 2, 2 * 128], F8, tag=f"w{g}", name=f"w{g}")
                if g < 2:
                    nc.sync.dma_start(t[:], wq_src(g))
                w_sb.append(t)
            bq_sb = const.tile([128, 3], F32, tag="bq")
            nc.scalar.dma_start(bq_sb[:], bq[:])
            for g in range(2, 4):
                nc.sync.dma_start(w_sb[g][:], wq_src(g))
            wv_sb = []
            for k in range(8):
                t = big.tile([128, 128], BF, tag=f"wv{k}", name=f"wv{k}")
                nc.sync.dma_start(t[:], wv[128 * k:128 * (k + 1), :])
                wv_sb.append(t)
            cos_sb = const.tile([128, L], BF, tag="cos")
            nc.scalar.dma_start(cos_sb[:], cosT[:])
            sin_sb = const.tile([128, L], BF, tag="sin")
            nc.scalar.dma_start(sin_sb[:], sinT[:])
            wo_sb = [big.tile([128, D], BF, tag=f"wo{k}", name=f"wo_{k}")
                     for k in range(8)]

            qT_sb = big.tile([128, T], BF, tag="qT")
            kT_sb = big.tile([128, T], BF, tag="kT")
            v_sb = big.tile([128, T], BF, tag="v")
            # transposed v with a built-in ones column: [tok%128, blk, head, 65]
            vn_sb = big.tile([128, T // 128, HPC, 65], BF, tag="vn")
            nc.vector.memset(vn_sb[:, :, :, 64:65], 1.0)

            # one AllToAll per batch: slot j = tokens [256j, 256j+256) of
            # that batch; batch-0's collective rides under batch-1 compute
            a2a_in = [dram.tile([NC, 128, HTOK], BF, tag=f"a2a_in{b}",
                                name=f"a2a_in{b}") for b in range(B)]
            a2a_out = [dram.tile([NC, 128, HTOK], BF, tag=f"a2a_out{b}",
                                 name=f"a2a_out{b}") for b in range(B)]

            # ---------------- per-stage emitters ------------------------------
            _xc_cache = {}

            def stage1_load(n):
                ts = slice(512 * n, 512 * (n + 1))
                xc = []
                for g in range(4):
                    t = rope.tile([128, 2, 512], F8, tag="xc", bufs=12,
                                  name=f"xc_{n}_{g}")
                    if n == 0:  # startup: keep the first x chunks off the
                        q = nc.scalar if g < 2 else nc.gpsimd  # busy sync q
                    else:
                        q = nc.sync
                    xt_ap = xT[:]
                    src8 = bass.AP(
                        tensor=xt_ap.tensor,
                        offset=256 * g * T + 512 * n,
                        ap=[[T, 128], [128 * T, 2], [1, 512]])
                    q.dma_start(t[:], src8)
                    xc.append(t)
                xb = []
                for k in range(8):
                    t = rope.tile([128, 512], BF, tag="xb", bufs=16,
                                  name=f"xb_{n}_{k}")
                    q = (nc.scalar if k < 4 else nc.gpsimd) if n == 0 \
                        else nc.sync
                    q.dma_start(t[:], xTb[128 * k:128 * (k + 1), ts])
                    xb.append(t)
                _xc_cache[n] = (xc, xb)

            _ps_cache = {}

            def stage1_qkv_m_a(n, m):
                """First half of the QKV accumulation for one (n-tile, m)."""
                ps = psum.tile([128, 512], F32, tag="st", bufs=3,
                               name=f"s1_{n}_{m}")
                _ps_cache[(n, m)] = ps
                xc, xb = _xc_cache[n]
                if m < 2:
                    for g in range(2):
                        nc.tensor.matmul(
                            ps[:],
                            w_sb[g][:, :, 128 * m:128 * (m + 1)],
                            xc[g][:],
                            start=(g == 0), stop=False,
                            perf_mode=mybir.MatmulPerfMode.DoubleRow,
                        )
                else:
                    for k in range(4):
                        nc.tensor.matmul(
                            ps[:], wv_sb[k][:], xb[k][:],
                            start=(k == 0), stop=False,
                        )

            def stage1_qkv_m(n, m):
                """Second half of the accumulation; ACT evicts (+bias), rope
                in bf16 split across DVE and GpSimd."""
                ts = slice(512 * n, 512 * (n + 1))
                cs = slice(512 * (n % QT), 512 * (n % QT) + 512)
                ps = _ps_cache.pop((n, m))
                xc, xb = _xc_cache[n]
                if m < 2:
                    for g in range(2, 4):
                        nc.tensor.matmul(
                            ps[:],
                            w_sb[g][:, :, 128 * m:128 * (m + 1)],
                            xc[g][:],
                            start=False, stop=(g == 3),
                            perf_mode=mybir.MatmulPerfMode.DoubleRow,
                        )
                else:
                    for k in range(4, 8):
                        nc.tensor.matmul(
                            ps[:], wv_sb[k][:], xb[k][:],
                            start=False, stop=(k == 7),
                        )
                if m < 2:  # q or k: ACT evicts (+bias) fast to free the
                    # PSUM slot; rope split across DVE and GpSimd
                    dst = qT_sb if m == 0 else kT_sb
                    qb = rope.tile([128, 512], BF, tag="qb", bufs=5,
                                   name=f"qb_{n}_{m}")
                    nc.scalar.activation(
                        qb[:], ps[:],
                        mybir.ActivationFunctionType.Identity,
                        bias=bq_sb[:, m:m + 1],
                        scale=1.0 / (WQ_SCALE * 8.0) if m == 0
                        else 1.0 / WQ_SCALE)
                    qc = rope.tile([128, 512], BF, tag="qc", name=f"qc_{n}_{m}")
                    nc.vector.tensor_tensor(
                        qc[:], qb[:], cos_sb[:, cs], mybir.AluOpType.mult)
                    qs = rope.tile([128, 512], BF, tag="qs", name=f"qs_{n}_{m}")
                    nc.vector.tensor_tensor(
                        qs[:], qb[:], sin_sb[:, cs], mybir.AluOpType.mult)
                    qw = rope.tile([128, 512], BF, tag="qw", name=f"qw_{n}_{m}")
                    for blk in range(4):
                        sb0 = 32 * (blk ^ 1)
                        nc.gpsimd.dma_start(
                            qw[32 * blk:32 * blk + 32, :],
                            qs[sb0:sb0 + 32, :])
                    nc.vector.tensor_tensor(
                        dst[:, ts], qc[:], qw[:], mybir.AluOpType.add)
                else:  # v: bias only, straight to bf16
                    nc.scalar.activation(
                        v_sb[:, ts], ps[:],
                        mybir.ActivationFunctionType.Identity,
                        bias=bq_sb[:, 2:3])

            def stage1_qkv(n):
                stage1_load(n)
                for m in range(3):
                    stage1_qkv_m_a(n, m)
                    stage1_qkv_m(n, m)

            def stage1_vtr(j):
                """Transpose one 128-token block of v into vn (both heads)."""
                tp = psum.tile([128, 128], BF, tag="st", bufs=3, name=f"tr_{j}")
                nc.tensor.transpose(tp[:], v_sb[:, 128 * j:128 * (j + 1)], ident[:])
                for h in range(HPC):
                    nc.vector.tensor_copy(
                        vn_sb[:, j, h, 0:64], tp[:, 64 * h:64 * (h + 1)])

            def stage2_open(b, qt):
                return [psum.tile([80, 512], F32, tag=f"ctx{h}", bufs=1,
                                  name=f"ctx_{b}_{qt}_{h}")
                        for h in range(HPC)]

            def stage2_kts(b, qt, ctxs, fill_boundary):
                qsl = slice(2048 * b + 512 * qt, 2048 * b + 512 * qt + 512)

                def emit_ctx(kt, es):
                    blk = 16 * b + kt
                    for h in range(HPC):
                        nc.tensor.matmul(
                            ctxs[h][0:65, :],
                            vn_sb[:, blk, h, :],
                            es[:, 512 * h:512 * (h + 1)],
                            start=(kt == 0), stop=(kt == KT - 1))

                def emit_ctx_pair(kt, es2):
                    bp = (16 * b + kt) // 2
                    for h in range(HPC):
                        nc.tensor.matmul(
                            ctxs[h][:],
                            vn2_sb[:, bp, h, :, :],
                            es2[:, :, 512 * h:512 * (h + 1)],
                            start=(kt == 0), stop=False,
                            perf_mode=mybir.MatmulPerfMode.DoubleRow)

                # chunked emission: runs of score-pairs (64-row PE mode, so
                # next pair's LDWEIGHTS pulls ahead into the idle row group)
                # alternate with runs of ctx matmuls + drip (128-row mode).
                # One chunk of software pipelining: chunk c's ctx is emitted
                # after chunk c+1's scores so exp has a full chunk of slack.
                # Chunks of 3 match the 3 "st" PSUM slots - a scores run
                # never waits on its own chunk's exp evictions.
                prev = []
                es2_cur = [None]
                for ch in CHUNKS:
                    cur = []
                    for kt in ch:
                        ksl = slice(2048 * b + 128 * kt,
                                    2048 * b + 128 * kt + 128)
                        st2 = psum.tile([128, 1024], F32, tag="st", bufs=3,
                                        name=f"st_{b}_{qt}_{kt}")
                        for h in range(HPC):
                            nc.tensor.matmul(
                                st2[:, 512 * h:512 * (h + 1)],
                                kT_sb[64 * h:64 * (h + 1), ksl],
                                qT_sb[64 * h:64 * (h + 1), qsl],
                                start=True, stop=True)
                        if kt in DR_KTS:
                            if kt % 2 == 0:
                                es2_cur[0] = esp.tile(
                                    [128, 2, 1024], F8, tag="es2",
                                    bufs=6, name=f"es2_{b}_{qt}_{kt}")
                            es2 = es2_cur[0]
                            nc.scalar.activation(
                                es2[:, kt % 2, :], st2[:],
                                mybir.ActivationFunctionType.Exp)
                            if kt % 2 == 1:
                                cur.append(("dr", kt - 1, es2))
                        elif kt in DVE_KTS[b]:
                            es = esp.tile([128, 1024], BF, tag="es",
                                          bufs=8, name=f"es_{b}_{qt}_{kt}")
                            nc.vector.tensor_scalar(
                                es[:].bitcast(I16), st2[:], EXP_A, EXP_B,
                                mybir.AluOpType.mult, mybir.AluOpType.add)
                            cur.append(("bf", kt, es))
                        else:
                            es = esp.tile([128, 1024], BF, tag="es",
                                          bufs=8, name=f"es_{b}_{qt}_{kt}")
                            nc.scalar.activation(
                                es[:], st2[:],
                                mybir.ActivationFunctionType.Exp)
                            cur.append(("bf", kt, es))
                    for kind, kt, e in prev:
                        if kind == "dr":
                            emit_ctx_pair(kt, e)
                        else:
                            emit_ctx(kt, e)
                    prev = cur
                    fill_boundary(b, qt, ch[0])
                for kind, kt, e in prev:
                    if kind == "dr":
                        emit_ctx_pair(kt, e)
                    else:
                        emit_ctx(kt, e)

            def a2a_write(b, qt, h, cn):
                # one 3D DMA moves both 256-token halves of cn into their
                # a2a slots (alternating issue queues - each DIRECT2D costs
                # ~600ns on its issuing engine)
                dst_t = a2a_in[b][:]
                slot = 128 * HTOK
                dst = bass.AP(
                    tensor=dst_t.tensor,
                    offset=dst_t.offset + (2 * qt) * slot + (64 * h) * HTOK,
                    ap=[[HTOK, 64], [slot, 2], [1, HTOK]])
                src_t = cn[:]
                src = bass.AP(
                    tensor=src_t.tensor, offset=src_t.offset,
                    ap=[list(src_t.ap[0]), [HTOK, 2], [1, HTOK]])
                (nc.sync if h == 0 else nc.scalar).dma_start(dst, src)

            def ctx_evict(b, qt, ctxs, pe_bcast=False):
                if pe_bcast:
                    # gpsimd-free eviction (batch 1 runs while the batch-0
                    # collective occupies the gpsimd queue): reciprocal
                    # broadcast over 64 partitions via a ones-matmul on the
                    # PE; both heads' chains interleaved
                    pre = {}
                    for h in range(HPC):
                        dn = small.tile([1, 512], F32, tag="dn",
                                        name=f"dnL_{b}_{qt}_{h}", bufs=3)
                        if h == 0:
                            nc.scalar.copy(dn[:], ctxs[h][64:65, :])
                        else:
                            nc.vector.tensor_copy(dn[:], ctxs[h][64:65, :])
                        rc = small.tile([1, 512], F32, tag="rc",
                                        name=f"rcL_{b}_{qt}_{h}", bufs=3)
                        nc.vector.reciprocal_approx_fast(rc[:], dn[:])
                        rcb = small.tile([1, 512], BF, tag="rcb",
                                         name=f"rcbL_{b}_{qt}_{h}", bufs=3)
                        nc.vector.tensor_copy(rcb[:], rc[:])
                        bcp = psum.tile([128, 512], F32, tag=f"ctx{h}",
                                        bufs=1, name=f"bcpL_{b}_{qt}_{h}")
                        nc.tensor.matmul(
                            bcp[0:64, :], ones_bc[:], rcb[:],
                            start=True, stop=True)
                        pre[h] = bcp
                    for h in range(HPC):
                        cu = cup.tile([65, 512], F32, tag="cu",
                                      name=f"cuL_{b}_{qt}_{h}")
                        if h == 0:
                            nc.scalar.copy(cu[:], ctxs[h][0:65, :])
                        else:
                            nc.vector.tensor_copy(cu[:], ctxs[h][0:65, :])
                        cn = small.tile([64, 512], BF, tag="cn", bufs=8,
                                        name=f"cnL_{b}_{qt}_{h}")
                        nc.vector.tensor_tensor(
                            cn[:], cu[0:64, :], pre[h][0:64, :],
                            mybir.AluOpType.mult)
                        a2a_write(b, qt, h, cn)
                    return
                for h in range(HPC):
                    eng = nc.scalar if h == 0 else nc.vector
                    cu = cup.tile([65, 512], F32, tag="cu",
                                  name=f"cu_{b}_{qt}_{h}")
                    dn = small.tile([1, 512], F32, tag="dn",
                                    name=f"dn_{b}_{qt}_{h}", bufs=3)
                    if h == 0:  # parallelize the two heads' chains
                        nc.scalar.copy(cu[:], ctxs[h][0:65, :])
                        nc.scalar.copy(dn[:], ctxs[h][64:65, :])
                    else:
                        nc.vector.tensor_copy(cu[:], ctxs[h][0:65, :])
                        nc.vector.tensor_copy(dn[:], ctxs[h][64:65, :])
                    rc = small.tile([1, 512], F32, tag="rc",
                                    name=f"rc_{b}_{qt}_{h}", bufs=3)
                    nc.vector.reciprocal_approx_fast(rc[:], dn[:])
                    # broadcast 1/den over 64 partitions via a stride-0
                    # DRAM read; runs entirely off the PE
                    dr = dram.tile([1, 512], F32, tag="dr",
                                   name=f"dr_{b}_{qt}_{h}", bufs=4)
                    nc.gpsimd.dma_start(dr[:], rc[:])
                    bca = small.tile([64, 512], F32, tag="bca",
                                     name=f"bca_{b}_{qt}_{h}", bufs=3)
                    dr_ap = dr[:]
                    bcast_src = bass.AP(
                        tensor=dr_ap.tensor, offset=dr_ap.offset,
                        ap=[[0, 32]] + [list(p) for p in dr_ap.ap])
                    nc.gpsimd.dma_start(bca[0:32, :], bcast_src)
                    nc.sync.dma_start(bca[32:64, :], bcast_src)
                    bca_ap = bca[:]
                    cn = small.tile([64, 512], BF, tag="cn", bufs=8,
                                    name=f"cn_{b}_{qt}_{h}")
                    nc.vector.tensor_tensor(
                        cn[:], cu[0:64, :], bca_ap,
                        mybir.AluOpType.mult)
                    a2a_write(b, qt, h, cn)

            def run_batch(b, fill_boundary):
                for qt in range(QT):
                    ctxs = stage2_open(b, qt)
                    stage2_kts(b, qt, ctxs, fill_boundary)
                    ctx_evict(b, qt, ctxs, pe_bcast=(b == 1))

            # dummy matmuls keep the PE HAM-warm when real work is thin;
            # short 2-matmul groups so a PSUM "st" slot is never held long
            dummy_scr = small.tile([1, 512], F32, tag="dscr", name="dscr",
                                   bufs=1)
            dummy_cnt = [0]

            def emit_dummies(n, w=512):
                for _ in range(n):
                    i = dummy_cnt[0]
                    dummy_cnt[0] += 1
                    if i % 2 == 0:
                        dummy_cnt.append(psum.tile(
                            [128, 512], F32, tag="st", bufs=3,
                            name=f"dmy{i}"))
                    dp = dummy_cnt[-1]
                    nc.tensor.matmul(
                        dp[:, 0:w], ident[:], kT_sb[:, 0:w],
                        start=(i % 2 == 0), stop=(i % 2 == 1))
                    if i % 2 == 1:
                        nc.vector.tensor_copy(
                            dummy_scr[:, 2 * ((i // 2) % 128):
                                      2 * ((i // 2) % 128) + 2],
                            dp[0:1, 0:2])

            # ---------------- emission schedule -------------------------------
            # batch-1 stage-1 work, cut into drip units
            b1_units = []
            for n in range(QT, NT):
                b1_units.append(lambda n=n: stage1_load(n))
                for m in range(3):
                    b1_units.append(lambda n=n, m=m: stage1_qkv_m_a(n, m))
                    b1_units.append(lambda n=n, m=m: stage1_qkv_m(n, m))
                for j in range(4 * n, 4 * n + 4):
                    b1_units.append(lambda j=j: stage1_vtr(j))
            unit_idx = [0]

            # stage 1 for batch 0 (transposes follow each n-tile's v)
            for n in range(QT):
                stage1_qkv(n)
                for j in range(4 * n, 4 * n + 4):
                    stage1_vtr(j)


            # 20 chunk boundaries in batch 0 carry the 44 batch-1 QKV
            # half-units, up to 3 per boundary, inside the full-mode runs
            def fill_b0(b, qt, kt):
                for _ in range(3):
                    if unit_idx[0] < len(b1_units):
                        b1_units[unit_idx[0]]()
                        unit_idx[0] += 1

            if dbg_on:
                dbt = small.tile([128, 1536], F32, tag="dbt", name="dbt")
                nc.vector.tensor_copy(dbt[:, 0:512], qT_sb[:, 0:512])
                nc.vector.tensor_copy(dbt[:, 512:1024], kT_sb[:, 0:512])
                nc.vector.tensor_copy(dbt[:, 1024:1536], v_sb[:, 0:512])
                nc.sync.dma_start(dbg[:], dbt[:])
            run_batch(0, fill_b0)
            # weights for the out projection: NOT on the gpsimd queue (the
            # collectives will occupy it through batch 1)
            for k in range(8):
                (nc.sync if k % 2 else nc.scalar).dma_start(
                    wo_sb[k][:], wo[128 * k:128 * (k + 1), :])
            nc.scalar.dma_start(bo_sb[:], bo[:])
            while unit_idx[0] < len(b1_units):
                b1_units[unit_idx[0]]()
                unit_idx[0] += 1

            # batch-0 AllToAll: trigger now, transfer rides under batch-1.
            # This is the LAST gpsimd work before the second collective -
            # batch 1 is entirely gpsimd-free.
            nc.gpsimd.collective_compute(
                "AllToAll",
                mybir.AluOpType.bypass,
                replica_groups=[list(range(NC))],
                ins=[a2a_in[0].opt()],
                outs=[a2a_out[0].opt()],
            )
            # batch-0 ctx gather: on the gpsimd queue, which the collective
            # just blocked anyway - executes the moment it lands, mid
            # batch-1, without stalling any other queue
            ctxf0_sb = []
            for k in range(8):
                t = big.tile([128, HTOK], BF, tag=f"cf0_{k}",
                             name=f"cf0_{k}")
                nc.gpsimd.dma_start(t[:], a2a_out[0][k, :, :])
                ctxf0_sb.append(t)

            def fill_b1(b, qt, kt):
                # occasional narrow dummy keeps the PE activity monitor at
                # full clock through batch 1
                if kt in (3, 9):
                    emit_dummies(1, w=128)

            run_batch(1, fill_b1)

            nc.gpsimd.collective_compute(
                "AllToAll",
                mybir.AluOpType.bypass,
                replica_groups=[list(range(NC))],
                ins=[a2a_in[1].opt()],
                outs=[a2a_out[1].opt()],
            )

            # ---------------- stage 4: out projection (per batch half) -------
            # batch-0's half runs on the PE while the batch-1 AllToAll is in
            # flight; batch-1's half follows when its data lands. m-outer
            # accumulation into 8 PSUM quarter-bank regions; bias via ACT.
            oslots = [psum.tile([128, 1024], F32, tag="st", bufs=3,
                                name=f"oacc{i}") for i in range(2)]
            accs = [oslots[i // 4][:, 256 * (i % 4):256 * (i % 4) + 256]
                    for i in range(8)]
            for bh in range(2):
                if bh == 0:
                    ctxf_sb = ctxf0_sb
                    dmaq = [nc.sync, nc.scalar]
                else:
                    ctxf_sb = []
                    ldq = [nc.sync, nc.scalar, nc.gpsimd]
                    for k in range(8):
                        t = big.tile([128, HTOK], BF, tag=f"cf1_{k}",
                                     name=f"cf1_{k}")
                        ldq[k % 3].dma_start(t[:], a2a_out[1][k, :, :])
                        ctxf_sb.append(t)
                    dmaq = [nc.sync, nc.gpsimd]
                for m in range(8):
                    for k in range(8):
                        nc.tensor.matmul(
                            accs[m],
                            wo_sb[k][:, 128 * m:128 * (m + 1)],
                            ctxf_sb[k][:],
                            start=(k == 0), stop=(k == 7))
                    os_t = small.tile([128, HTOK], BF, tag="os",
                                      name=f"os_{bh}_{m}", bufs=6)
                    nc.scalar.activation(  # ACT idle here: evict+bias in one
                        os_t[:], accs[m],
                        mybir.ActivationFunctionType.Identity,
                        bias=bo_sb[:, m:m + 1])
                    dmaq[m % 2].dma_start(
                        out[128 * m:128 * (m + 1), HTOK * bh:HTOK * (bh + 1)],
                        os_t[:])
                if bh == 0:
                    # keep the PE activity monitor warm across the remainder
                    # of the batch-1 collective
                    emit_dummies(6, w=128)


    nc.compile()
    return nc


_NC_CACHE = None


def _get_nc():
    global _NC_CACHE
    if _NC_CACHE is None:
        _NC_CACHE = build()
    return _NC_CACHE


def _host_prep(x, W_qkv, b_qkv, W_out, b_out):
    x = np.asarray(x, dtype=np.float32)
    W_qkv = np.asarray(W_qkv, dtype=np.float32)
    b_qkv = np.asarray(b_qkv, dtype=np.float32)
    W_out = np.asarray(W_out, dtype=np.float32)
    b_out = np.asarray(b_out, dtype=np.float32)

    scale = 1.0 / np.sqrt(Hd)
    xTb = np.ascontiguousarray(x.reshape(T, D).T).astype(BF16)
    xT = xTb.astype(FP8E4)

    # rope tables (token position within batch), channel-transposed + sign-folded
    inv_freq = 1.0 / (10000.0 ** (np.arange(0, Hd, 2, dtype=np.float32) / Hd))  # [32]
    t_pos = np.arange(L, dtype=np.float32)
    freqs = np.outer(t_pos, inv_freq)                       # [L, 32]
    emb = np.concatenate([freqs, freqs], axis=1)            # [L, 64]
    cos_t = np.cos(emb).T.astype(np.float32)                # [64, L]
    sin_t = np.sin(emb).T.astype(np.float32)                # [64, L]
    sin2 = sin_t.copy()
    sin2[32:, :] *= -1.0                                    # s''[d] = +sin d<32, -sin d>=32
    cosT = np.ascontiguousarray(np.tile(cos_t, (2, 1))).astype(BF16)  # [128, L]
    sinT = np.ascontiguousarray(np.tile(sin2, (2, 1))).astype(BF16)

    woutT = np.ascontiguousarray(W_out.T).astype(BF16)      # [D, D]
    bo_sb = np.ascontiguousarray(b_out.reshape(NC, 128).T.copy()).astype(np.float32)  # [128, 8]

    in_maps = []
    for c in range(NC):
        r = slice(128 * c, 128 * (c + 1))
        Wq = W_qkv[0 * D:1 * D][r] * (scale * 8.0)  # x8 keeps fp8 normal;
        # the q eviction unscales by WQ_SCALE*8
        Wk = W_qkv[1 * D:2 * D][r]
        Wv = W_qkv[2 * D:3 * D][r]
        Wc = np.concatenate([Wq, Wk], axis=0)               # [256, 1024]
        WcT = np.ascontiguousarray(Wc.T * WQ_SCALE).astype(FP8E4)  # [1024, 256]
        WvT = np.ascontiguousarray(Wv.T).astype(BF16)       # [1024, 128]
        bq_c = np.stack([
            b_qkv[0 * D:1 * D][r] * scale,
            b_qkv[1 * D:2 * D][r],
            b_qkv[2 * D:3 * D][r],
        ], axis=1).astype(np.float32)                       # [128, 3]
        in_maps.append({
            "xT": xT,
            "xTb": xTb,
            "wqkT": WcT,
            "wvT": WvT,
            "bqkv": np.ascontiguousarray(bq_c),
            "cosT": cosT,
            "sinT": sinT,
            "woutT": woutT,
            "bout": bo_sb,
        })
    return in_maps


def kernel_run(inputs, trace=False, tmpdir=None):
    nc = _get_nc()
    in_maps = _host_prep(**inputs)
    res = run_bass_kernel_spmd(
        nc, in_maps, list(range(NC)), trace=trace, tmpdir=tmpdir)
    # core c returns [D, 512]: cols 0-255 = batch-0 tokens [256c, 256c+256),
    # cols 256-511 = the same token range of batch 1
    outT = np.empty((D, T), dtype=np.float32)
    for c in range(NC):
        o = np.asarray(res.results[c]["out"]).astype(np.float32)
        outT[:, HTOK * c:HTOK * (c + 1)] = o[:, :HTOK]
        outT[:, L + HTOK * c:L + HTOK * (c + 1)] = o[:, HTOK:]
    out = np.ascontiguousarray(outT.T).reshape(B, L, D)
    return out, res


def kernel(**inputs):
    out, _ = kernel_run(inputs, trace=False)
    return out

